# revision 17
# baseline (speedup 1.0000x reference)
# Nystrom attention TRN2 kernel (B=4, N=4096, D=768, H=12, m=64 landmarks).
#
# Sharding: 8 cores; core c handles batch b = c//2 and a 6-head group
# hg = c%2 (heads 6*hg .. 6*hg+5, organized as 3 adjacent pairs).
# Each core computes its heads' full contribution through w_proj plus half
# the proj bias; the host unshards by summing the two partials per batch.
#
# Per-core pipeline (all matmuls on PE, fp32 data, fp32r perf mode where the
# moving free dim is large):
#   A: stream x in 512-row blocks, PE-transpose to get D-major X^T, then
#      QT/KT head-major (pair-stacked on partitions) resident in SBUF and
#      V row-major streamed to a DRAM scratch buffer.
#   B: landmark gathers (stride-65 column slices), S2 -> K2 -> A = K2+eps*I,
#      Newton-Schulz inverse (34 iters, first 20 in bf16, dual X/Z iterate,
#      2-head block-diagonal batching).
#   C: S3 m-major (block-diag landmark lhsT), exp, PE-transpose chunks,
#      E3V accumulation, normalize by r3 -> K3V.
#   GT/GW: G^T = K3V^T Z, GW_h = G_h @ Wp_h (m x 768, pair-stacked).
#   D: per 128-row chunk: S1 row-major, exp, row-normalize, PE-transpose,
#      fused (K1 @ GW) projection, +bias/2, DMA out.

import numpy as np

B, N, D = 4, 4096, 768
H, M, HD = 12, 64, 64
HG = 6          # heads per core
PAIRS = 3       # head pairs per core
P = 128
NBLK = 256      # phase A/C n-block
NB = N // NBLK  # 8
NCH = N // P    # 32
SCALE = 0.125   # hd^-0.5
EPS = 1e-6
NS_ITERS = 34
NS_BF16 = 20
LSTRIDE = 65    # landmark stride: linspace(0,4095,64) == 65*arange(64)

_PROGRAM = None
DEBUG_PHASE = 0  # 0=full, 1=A only, 2=A+B, 3=A+B+NS, 4=+C, 5=+GT/GW


def _build_program():
    import concourse.bass as bass
    import concourse.mybir as mybir
    import concourse.tile as tile
    from concourse import bacc
    from concourse.masks import make_identity
    from contextlib import ExitStack

    DT = mybir.dt.float32
    BF = mybir.dt.bfloat16
    F32R = mybir.dt.float32r
    EXP = mybir.ActivationFunctionType.Exp
    IDENT = mybir.ActivationFunctionType.Identity
    AX = mybir.AxisListType.X

    def r32(ap):
        return ap.bitcast(F32R)

    nc = bacc.Bacc(trn_type="TRN2", target_bir_lowering=False, debug=False)

    x_d = nc.dram_tensor("x", [N, D], F32R, kind="ExternalInput")
    wqk_d = nc.dram_tensor("wqk", [D, 2 * HG * HD], F32R, kind="ExternalInput")
    bqk_d = nc.dram_tensor("bqk", [2 * HG * HD], DT, kind="ExternalInput")
    wv_d = nc.dram_tensor("wv", [D, HG * HD], F32R, kind="ExternalInput")
    bv_d = nc.dram_tensor("bv", [HG * HD], DT, kind="ExternalInput")
    wp_d = nc.dram_tensor("wp", [HG * HD, D], F32R, kind="ExternalInput")
    bph_d = nc.dram_tensor("bph", [D], DT, kind="ExternalInput")
    xl_d = nc.dram_tensor("xl", [M, D], DT, kind="ExternalInput")
    wqk32_d = nc.dram_tensor("wqk32", [D, 2 * HG * HD], DT, kind="ExternalInput")
    out_d = nc.dram_tensor("out", [N, D], DT, kind="ExternalOutput")

    with tile.TileContext(nc) as tc, ExitStack() as ctx:
        singles = ctx.enter_context(tc.tile_pool(name="singles", bufs=1))
        res = ctx.enter_context(tc.tile_pool(name="res", bufs=1))
        dram = ctx.enter_context(tc.tile_pool(name="dram", bufs=1, space="DRAM"))

        ident = singles.tile([P, P], DT)
        make_identity(nc, ident)
        identr = singles.tile([P, P], F32R)
        nc.vector.tensor_copy(identr, ident)
        twoI = singles.tile([P, P], DT)
        nc.vector.tensor_scalar_mul(twoI, ident, 2.0)
        epsI = singles.tile([P, M], DT)  # [eps*I64 ; eps*I64] stacked
        nc.vector.tensor_scalar_mul(epsI[0:M, :], ident[0:M, 0:M], EPS)
        nc.vector.tensor_scalar_mul(epsI[M:P, :], ident[M:P, M:P], EPS)

        biasqk = singles.tile([P, 2 * PAIRS], DT)
        nc.sync.dma_start(out=biasqk, in_=bqk_d.ap().rearrange("(c p) -> p c", p=P))
        bv_bc = singles.tile([P, HG * HD], DT)
        bv_ap = bv_d.ap()
        nc.sync.dma_start(
            out=bv_bc,
            in_=bass.AP(tensor=bv_ap.tensor, offset=bv_ap.offset,
                        ap=[[0, P], [1, HG * HD]]),
        )
        bp_bc = singles.tile([P, D], DT)
        bp_ap = bph_d.ap()
        nc.sync.dma_start(
            out=bp_bc,
            in_=bass.AP(tensor=bp_ap.tensor, offset=bp_ap.offset,
                        ap=[[0, P], [1, D]]),
        )
        wqk_sb = singles.tile([P, 6, 2 * HG * HD], F32R)
        nc.sync.dma_start(out=wqk_sb, in_=wqk_d.ap().rearrange("(c p) f -> p c f", p=P))
        wv_sb = singles.tile([P, 6, HG * HD], F32R)
        nc.sync.dma_start(out=wv_sb, in_=wv_d.ap().rearrange("(c p) f -> p c f", p=P))
        wp_sb = singles.tile([M, HG, D], F32R)
        nc.sync.dma_start(out=wp_sb, in_=wp_d.ap().rearrange("(h p) f -> p h f", p=M))

        QT = res.tile([P, PAIRS, N], F32R)   # partitions = pair-stacked head dims
        KT = res.tile([P, PAIRS, N], F32R)
        gw_sb = res.tile([P, PAIRS, D], F32R)
        v_dram = dram.tile([N, HG * HD], F32R)

        # ---------------- Phase A: qkv projection ----------------
        with tc.tile_pool(name="pa", bufs=2) as pa, \
             tc.tile_pool(name="pa_vt", bufs=3) as pavt, \
             tc.tile_pool(name="pap_t", bufs=3, space="PSUM") as papt, \
             tc.tile_pool(name="pap_qk", bufs=2, space="PSUM") as papqk, \
             tc.tile_pool(name="pap_v", bufs=2, space="PSUM") as papv:
            for nb in range(NB):
                nsl = slice(nb * NBLK, (nb + 1) * NBLK)
                xin = pa.tile([P, NBLK // P, D], F32R, tag="xin")
                nc.sync.dma_start(
                    out=xin, in_=x_d.ap()[nsl, :].rearrange("(c p) d -> p c d", p=P))
                xt = pa.tile([P, 6, NBLK], F32R, tag="xt")
                for c4 in range(NBLK // P):
                    for dc in range(6):
                        tp = papt.tile([P, P], F32R, tag="tp")
                        nc.tensor.transpose(
                            tp, xin[:, c4, dc * P:(dc + 1) * P], identr)
                        nc.any.tensor_copy(
                            out=xt[:, dc, c4 * P:(c4 + 1) * P],
                            in_=tp.bitcast(DT))
                if DEBUG_PHASE == 7 and nb == 0:
                    with tc.tile_pool(name="dbg7", bufs=1) as dbg7:
                        t7 = dbg7.tile([P, D], DT)
                        for dc in range(3):
                            nc.vector.tensor_copy(
                                t7[:, dc * NBLK:(dc + 1) * NBLK],
                                xt[:, dc, :].bitcast(DT))
                        nc.sync.dma_start(out=out_d.ap()[0:P, :], in_=t7)
                        t8 = dbg7.tile([P, D], DT)
                        nc.vector.tensor_copy(t8[:, 0:D], xin[:, 0, :].bitcast(DT))
                        nc.sync.dma_start(out=out_d.ap()[P:2 * P, :], in_=t8)
                for oc in range(6):
                    ps = papqk.tile([P, NBLK], DT, tag="qk")
                    for dc in range(6):
                        nc.tensor.matmul(
                            ps, wqk_sb[:, dc, oc * P:(oc + 1) * P],
                            xt[:, dc, :],
                            start=(dc == 0), stop=(dc == 5))
                    dest = QT if oc < 3 else KT
                    col = oc if oc < 3 else oc - 3
                    nc.scalar.activation(
                        out=dest[:, col, nsl], in_=ps, func=IDENT,
                        bias=biasqk[:, oc:oc + 1], scale=1.0)
                for c4 in range(NBLK // P):
                    psv = papv.tile([P, HG * HD], DT, tag="v")
                    for dc in range(6):
                        nc.tensor.matmul(
                            psv, xt[:, dc, c4 * P:(c4 + 1) * P],
                            wv_sb[:, dc, :],
                            start=(dc == 0), stop=(dc == 5))
                    vt = pavt.tile([P, HG * HD], F32R, tag="vt")
                    nc.vector.tensor_add(vt, psv, bv_bc)
                    nc.sync.dma_start(
                        out=v_dram[(nb * (NBLK // P) + c4) * P:(nb * (NBLK // P) + c4 + 1) * P, :],
                        in_=vt)

        if DEBUG_PHASE == 1:
            with tc.tile_pool(name="dbg", bufs=2) as dbg:
                for ncx in range(NCH):
                    t = dbg.tile([P, D], DT, tag="dbg")
                    nc.vector.memset(t, 0.0)
                    nc.vector.tensor_copy(t[:, 0:128], QT[:, 0, ncx * P:(ncx + 1) * P].bitcast(DT))
                    nc.vector.tensor_copy(t[:, 128:256], KT[:, 0, ncx * P:(ncx + 1) * P].bitcast(DT))
                    nc.sync.dma_start(out=out_d.ap()[ncx * P:(ncx + 1) * P, :], in_=t)

        # ---------------- Phase B: landmarks, A matrices ----------------
        RUN = lambda k: DEBUG_PHASE == 0 or DEBUG_PHASE >= k
        pb = ctx.enter_context(tc.tile_pool(name="pb", bufs=1))
        with tc.tile_pool(name="pbp", bufs=2, space="PSUM") as pbp, \
             tc.tile_pool(name="pbt", bufs=1) as pbt:
            # fp32 landmark path: A = K2 + eps*I must be fp32-exact because
            # cond(A) ~ 2e4 amplifies f32r rounding in the inverse.
            qlt_blk = pb.tile([P, PAIRS, P], F32R)  # block-diag Q_l^T per pair
            klt_blk = pb.tile([P, PAIRS, P], F32R)  # block-diag K_l^T per pair
            nc.vector.memset(qlt_blk.bitcast(DT), 0.0)
            nc.vector.memset(klt_blk.bitcast(DT), 0.0)
            xl = pbt.tile([M, D], DT)
            nc.sync.dma_start(out=xl, in_=xl_d.ap())
            wqk32 = pbt.tile([P, 6, 2 * HG * HD], DT)
            nc.sync.dma_start(
                out=wqk32,
                in_=wqk32_d.ap().rearrange("(c p) f -> p c f", p=P))
            xlt = pbt.tile([P, 6, M], DT)
            for dc in range(6 if RUN(2) else 0):
                tx = pbp.tile([P, M], DT, tag="xlt")
                nc.tensor.transpose(tx, xl[:, dc * P:(dc + 1) * P],
                                    ident[0:M, 0:M])
                nc.any.tensor_copy(out=xlt[:, dc, :], in_=tx)
            qkl32 = pbt.tile([P, 6, M], DT)  # oc 0-2: Q_l^T pairs, 3-5: K_l^T
            for oc in range(6 if RUN(2) else 0):
                pql = pbp.tile([P, M], DT, tag="pql")
                for dc in range(6):
                    nc.tensor.matmul(pql, wqk32[:, dc, oc * P:(oc + 1) * P],
                                     xlt[:, dc, :],
                                     start=(dc == 0), stop=(dc == 5))
                nc.scalar.activation(out=qkl32[:, oc, :], in_=pql, func=IDENT,
                                     bias=biasqk[:, oc:oc + 1], scale=1.0)
            qlt_blk32 = pbt.tile([P, PAIRS, P], DT)
            nc.vector.memset(qlt_blk32, 0.0)
            for pr in range(PAIRS if RUN(2) else 0):
                nc.any.tensor_copy(out=klt_blk[0:M, pr, 0:M],
                                   in_=qkl32[0:M, 3 + pr, :])
                nc.any.tensor_copy(out=klt_blk[M:P, pr, M:P],
                                   in_=qkl32[M:P, 3 + pr, :])
                nc.any.tensor_copy(out=qlt_blk[0:M, pr, 0:M],
                                   in_=qkl32[0:M, pr, :])
                nc.any.tensor_copy(out=qlt_blk[M:P, pr, M:P],
                                   in_=qkl32[M:P, pr, :])
                nc.any.tensor_copy(out=qlt_blk32[0:M, pr, 0:M],
                                   in_=qkl32[0:M, pr, :])
                nc.any.tensor_copy(out=qlt_blk32[M:P, pr, M:P],
                                   in_=qkl32[M:P, pr, :])
            A_st = pb.tile([P, PAIRS, M], DT)     # pair-stacked A = K2 + eps*I
            r2 = pb.tile([P, PAIRS], DT)
            for pr in range(PAIRS if RUN(2) else 0):
                ps2 = pbp.tile([P, M], DT, tag="s2")
                nc.tensor.matmul(ps2, qlt_blk32[:, pr, :], qkl32[:, 3 + pr, :],
                                 start=True, stop=True)
                e2 = pb.tile([P, M], DT, tag=f"e2_{pr}")
                nc.scalar.activation(out=e2, in_=ps2, func=EXP, scale=SCALE,
                                     accum_out=r2[:, pr:pr + 1])
                r2c = pb.tile([P, 1], DT, tag=f"r2c_{pr}")
                nc.vector.reciprocal(r2c, r2[:, pr:pr + 1])
                nc.vector.tensor_scalar_mul(A_st[:, pr, :], e2, r2c)
                nc.vector.tensor_add(A_st[:, pr, :], A_st[:, pr, :], epsI)

        if DEBUG_PHASE == 2:
            with tc.tile_pool(name="dbg", bufs=2) as dbg:
                t = dbg.tile([P, D], DT, tag="dbg")
                nc.vector.memset(t, 0.0)
                nc.vector.tensor_copy(t[:, 0:3 * M], A_st.bitcast(DT) if A_st.dtype != DT else A_st)
                nc.sync.dma_start(out=out_d.ap()[0:P, :], in_=t)

        # ---------------- Newton-Schulz inverse (dual iterate) ----------------
        ns_x = []
        ns_z = []
        with tc.tile_pool(name="nsp", bufs=2, space="PSUM") as nsp:
            for pr in range(PAIRS if RUN(3) else 0):
                Ablk = pb.tile([P, P], DT, tag=f"ablk{pr}")
                nc.vector.memset(Ablk, 0.0)
                nc.any.tensor_copy(out=Ablk[0:M, 0:M], in_=A_st[0:M, pr, :])
                nc.any.tensor_copy(out=Ablk[M:P, M:P], in_=A_st[M:P, pr, :])
                tb = nsp.tile([P, P], DT, tag="ns")
                nc.tensor.transpose(tb, Ablk, ident)  # A^T blockdiag
                b_f = pb.tile([P, P], DT, tag=f"bf{pr}")
                nc.any.tensor_copy(out=b_f, in_=tb)
                b_bf = pb.tile([P, P], BF, tag=f"bbf{pr}")
                nc.any.tensor_copy(out=b_bf, in_=tb)
                x_bf = pb.tile([P, P], BF, tag=f"xbf{pr}")
                nc.any.tensor_copy(out=x_bf, in_=tb)        # X0 = A^T (a0=1)
                z_bf = pb.tile([P, P], BF, tag=f"zbf{pr}")
                nc.any.tensor_copy(out=z_bf, in_=Ablk)      # Z0 = X0^T = A
                x_f = pb.tile([P, P], DT, tag=f"xf{pr}")
                z_f = pb.tile([P, P], DT, tag=f"zf{pr}")
                for it in range(NS_ITERS):
                    bf_now = it < NS_BF16
                    bf_next = (it + 1) < NS_BF16
                    cx = x_bf if bf_now else x_f
                    cz = z_bf if bf_now else z_f
                    cb = b_bf if bf_now else b_f
                    t1 = nsp.tile([P, P], DT, tag="ns")
                    nc.tensor.matmul(t1, cb, cx, start=True, stop=True)  # A@X
                    u = pb.tile([P, P], BF if bf_now else DT,
                                tag=f"u{pr}_{it % 2}_{int(bf_now)}")
                    nc.vector.tensor_sub(u, twoI, t1)        # U = 2I - A X
                    xn = nsp.tile([P, P], DT, tag="ns")
                    nc.tensor.matmul(xn, cz, u, start=True, stop=True)   # X@U
                    zn = nsp.tile([P, P], DT, tag="ns")
                    nc.tensor.matmul(zn, u, cz, start=True, stop=True)   # U^T@Z
                    if bf_next:
                        nc.any.tensor_copy(out=x_bf, in_=xn)
                        nc.any.tensor_copy(out=z_bf, in_=zn)
                    else:
                        nc.any.tensor_copy(out=x_f, in_=xn)
                        nc.any.tensor_copy(out=z_f, in_=zn)
                ns_x.append(x_f)
                ns_z.append(z_f)

        if DEBUG_PHASE == 3:
            with tc.tile_pool(name="dbg", bufs=2) as dbg:
                t = dbg.tile([P, D], DT, tag="dbg")
                nc.vector.memset(t, 0.0)
                for pr in range(PAIRS):
                    nc.vector.tensor_copy(t[:, pr * P:(pr + 1) * P], ns_z[pr])
                nc.sync.dma_start(out=out_d.ap()[0:P, :], in_=t)

        # ---------------- Phase C: S3, E3V, K3V ----------------
        k3v_st = pb.tile([P, PAIRS, M], DT)
        with tc.tile_pool(name="pc", bufs=3) as pc, \
             tc.tile_pool(name="pcp_s3", bufs=2, space="PSUM") as pcps3, \
             tc.tile_pool(name="pcp_t", bufs=2, space="PSUM") as pcpt, \
             tc.tile_pool(name="pcp_acc", bufs=1, space="PSUM") as pcpacc:
            r3acc = pb.tile([P, PAIRS, NB], DT)
            k3vps = pcpacc.tile([P, PAIRS, P], DT)  # one bank, 3 accum regions
            for nb in range(NB if RUN(4) else 0):
                nsl = slice(nb * NBLK, (nb + 1) * NBLK)
                vblk = pc.tile([P, NBLK // P, HG * HD], F32R, tag="vblk")
                nc.sync.dma_start(
                    out=vblk,
                    in_=v_dram[nsl, :].rearrange("(c p) f -> p c f", p=P))
                for pr in range(PAIRS):
                    s3 = pcps3.tile([P, NBLK], DT, tag="s3")
                    nc.tensor.matmul(s3, qlt_blk[:, pr, :],
                                     KT[:, pr, nsl], start=True, stop=True)
                    e3 = pc.tile([P, NBLK], F32R, tag="e3")
                    nc.scalar.activation(out=e3, in_=s3, func=EXP, scale=SCALE,
                                         accum_out=r3acc[:, pr, nb:nb + 1])
                    for c4 in range(NBLK // P):
                        tp = pcpt.tile([P, P], F32R, tag="e3t")
                        nc.tensor.transpose(
                            tp, e3[:, c4 * P:(c4 + 1) * P], identr)
                        e3t = pc.tile([P, P], F32R, tag="e3ts")
                        nc.any.tensor_copy(out=e3t, in_=tp.bitcast(DT))
                        first = (nb == 0 and pr == 0 and c4 == 0)
                        last = (nb == NB - 1 and c4 == NBLK // P - 1)
                        nc.tensor.matmul(
                            k3vps[:, pr, :], e3t,
                            vblk[:, c4, pr * P:(pr + 1) * P],
                            start=first, stop=last, skip_group_check=True)
            for pr in range(PAIRS if RUN(4) else 0):
                r3 = pb.tile([P, 1], DT, tag=f"r3_{pr}")
                nc.vector.reduce_sum(r3, r3acc[:, pr, :], axis=AX)
                r3c = pb.tile([P, 1], DT, tag=f"r3c_{pr}")
                nc.vector.reciprocal(r3c, r3)
                for h2 in range(2):
                    sl = slice(h2 * M, (h2 + 1) * M)
                    nc.vector.tensor_scalar_mul(
                        k3v_st[sl, pr, :], k3vps[sl, pr, h2 * M:(h2 + 1) * M],
                        r3c[sl])

        if DEBUG_PHASE == 4:
            with tc.tile_pool(name="dbg", bufs=2) as dbg:
                t = dbg.tile([P, D], DT, tag="dbg")
                nc.vector.memset(t, 0.0)
                nc.vector.tensor_copy(t[:, 0:3 * M], k3v_st)
                nc.sync.dma_start(out=out_d.ap()[0:P, :], in_=t)

        # ---------------- GT / GW ----------------
        with tc.tile_pool(name="pg", bufs=1) as pg, \
             tc.tile_pool(name="pgp", bufs=2, space="PSUM") as pgp:
            for pr in range(PAIRS if RUN(5) else 0):
                gtp = pgp.tile([M, P], DT, tag="gt")
                nc.tensor.matmul(gtp, k3v_st[:, pr, :], ns_z[pr],
                                 start=True, stop=True)  # [GT_h1 | GT_h2]
                gt = pg.tile([M, P], F32R, tag=f"gt{pr}")
                nc.any.tensor_copy(out=gt, in_=gtp)
                for h2 in range(2):
                    lt = gt[:, h2 * M:(h2 + 1) * M]
                    rh = wp_sb[:, 2 * pr + h2, :]
                    g1 = pgp.tile([M, 512], DT, tag="gw1")
                    nc.tensor.matmul(g1, lt, rh[:, 0:512],
                                     start=True, stop=True)
                    g2 = pgp.tile([M, 256], DT, tag="gw2")
                    nc.tensor.matmul(g2, lt, rh[:, 512:768],
                                     start=True, stop=True)
                    nc.any.tensor_copy(
                        out=gw_sb[h2 * M:(h2 + 1) * M, pr, 0:512], in_=g1)
                    nc.any.tensor_copy(
                        out=gw_sb[h2 * M:(h2 + 1) * M, pr, 512:768], in_=g2)

        if DEBUG_PHASE == 5:
            with tc.tile_pool(name="dbg", bufs=2) as dbg:
                t = dbg.tile([P, D], DT, tag="dbg")
                nc.vector.tensor_copy(t, gw_sb[:, 0, :].bitcast(DT))
                nc.sync.dma_start(out=out_d.ap()[0:P, :], in_=t)

        # ---------------- Phase D: S1, K1, fused projection ----------------
        with tc.tile_pool(name="pd", bufs=3) as pd, \
             tc.tile_pool(name="pd_k1", bufs=2) as pdk1, \
             tc.tile_pool(name="pdp_s1", bufs=2, space="PSUM") as pdps1, \
             tc.tile_pool(name="pdp_t", bufs=2, space="PSUM") as pdpt, \
             tc.tile_pool(name="pdp_o", bufs=2, space="PSUM") as pdpo:
            ncount = NCH if DEBUG_PHASE != 8 else 1
            for ncx in range(ncount if RUN(6) else 0):
                nsl = slice(ncx * P, (ncx + 1) * P)
                s1 = pdps1.tile([P, HG * M], DT, tag="s1")
                for pr in range(PAIRS):
                    nc.tensor.matmul(
                        s1[:, pr * P:(pr + 1) * P],
                        QT[:, pr, nsl], klt_blk[:, pr, :],
                        start=(pr == 0), stop=(pr == PAIRS - 1),
                        skip_group_check=True)
                e1 = pd.tile([P, HG * M], DT, tag="e1")
                nc.scalar.activation(out=e1, in_=s1, func=EXP, scale=SCALE)
                r1 = pd.tile([P, HG], DT, tag="r1")
                nc.vector.reduce_sum(
                    r1, e1.rearrange("p (h m) -> p h m", h=HG), axis=AX)
                rc = pd.tile([P, HG], DT, tag="rc")
                nc.vector.reciprocal(rc, r1)
                e1n = pd.tile([P, HG * M], F32R, tag="e1n")
                for h6 in range(HG):
                    nc.vector.tensor_scalar_mul(
                        e1n[:, h6 * M:(h6 + 1) * M],
                        e1[:, h6 * M:(h6 + 1) * M], rc[:, h6:h6 + 1])
                k1ts = []
                for kc in range(PAIRS):
                    tp = pdpt.tile([P, P], F32R, tag="k1tp")
                    nc.tensor.transpose(
                        tp, e1n[:, kc * P:(kc + 1) * P], identr)
                    k1t = pdk1.tile([P, P], F32R, tag=f"k1t{kc}")
                    nc.any.tensor_copy(out=k1t, in_=tp.bitcast(DT))
                    k1ts.append(k1t)
                po1 = pdpo.tile([P, 512], DT, tag="po1")
                po2 = pdpo.tile([P, 256], DT, tag="po2")
                for kc in range(PAIRS):
                    nc.tensor.matmul(po1, k1ts[kc], gw_sb[:, kc, 0:512],
                                     start=(kc == 0), stop=(kc == PAIRS - 1))
                    nc.tensor.matmul(po2, k1ts[kc], gw_sb[:, kc, 512:768],
                                     start=(kc == 0), stop=(kc == PAIRS - 1))
                ob = pd.tile([P, D], DT, tag="ob")
                nc.vector.tensor_add(ob[:, 0:512], po1, bp_bc[:, 0:512])
                nc.vector.tensor_add(ob[:, 512:768], po2, bp_bc[:, 512:768])
                nc.sync.dma_start(out=out_d.ap()[nsl, :], in_=ob)

    nc.compile()
    return nc


def get_program():
    global _PROGRAM
    if _PROGRAM is None:
        _PROGRAM = _build_program()
    return _PROGRAM


def shard_inputs(x, w_qkv, b_qkv, w_proj, b_proj):
    """Returns one in_map per core (core = 2*b + head_group)."""
    x = np.ascontiguousarray(x, dtype=np.float32)
    w_qkv = np.asarray(w_qkv, dtype=np.float32)
    b_qkv = np.asarray(b_qkv, dtype=np.float32)
    w_proj = np.asarray(w_proj, dtype=np.float32)
    b_proj = np.asarray(b_proj, dtype=np.float32)
    wq, wk, wv = w_qkv[:, 0:D], w_qkv[:, D:2 * D], w_qkv[:, 2 * D:3 * D]
    bq, bk, bv = b_qkv[0:D], b_qkv[D:2 * D], b_qkv[2 * D:3 * D]
    in_maps = []
    for core in range(8):
        b, hg = core // 2, core % 2
        cs = slice(hg * HG * HD, (hg + 1) * HG * HD)
        wqk_c = np.ascontiguousarray(
            np.concatenate([wq[:, cs], wk[:, cs]], axis=1))
        in_maps.append({
            "x": np.ascontiguousarray(x[b]),
            "wqk": wqk_c,
            "xl": np.ascontiguousarray(x[b][0:4096:65, :]),
            "wqk32": wqk_c.copy(),
            "bqk": np.ascontiguousarray(
                np.concatenate([bq[cs], bk[cs]])),
            "wv": np.ascontiguousarray(wv[:, cs]),
            "bv": np.ascontiguousarray(bv[cs]),
            "wp": np.ascontiguousarray(w_proj[cs.start:cs.stop, :]),
            "bph": np.ascontiguousarray(0.5 * b_proj),
        })
    return in_maps


def run_cores(in_maps, trace=False, **kw):
    from concourse import bass_utils
    nc = get_program()
    return bass_utils.run_bass_kernel_spmd(
        nc, in_maps, core_ids=list(range(8)), trace=trace, **kw)


def unshard_output(results):
    out = np.empty((B, N, D), dtype=np.float32)
    for b in range(B):
        out[b] = results[2 * b]["out"] + results[2 * b + 1]["out"]
    return out


def kernel(x, w_qkv, b_qkv, w_proj, b_proj):
    in_maps = shard_inputs(x, w_qkv, b_qkv, w_proj, b_proj)
    res = run_cores(in_maps)
    return unshard_output(res.results)


# revision 19
# speedup vs baseline: 1.0002x; 1.0002x over previous
# Nystrom attention TRN2 kernel (B=4, N=4096, D=768, H=12, m=64 landmarks).
#
# Sharding: 8 cores; core c handles batch b = c//2 and a 6-head group
# hg = c%2 (heads 6*hg .. 6*hg+5, organized as 3 adjacent pairs).
# Each core computes its heads' full contribution through w_proj plus half
# the proj bias; the host unshards by summing the two partials per batch.
#
# Per-core pipeline (all matmuls on PE, fp32 data, fp32r perf mode where the
# moving free dim is large):
#   A: stream x in 512-row blocks, PE-transpose to get D-major X^T, then
#      QT/KT head-major (pair-stacked on partitions) resident in SBUF and
#      V row-major streamed to a DRAM scratch buffer.
#   B: landmark gathers (stride-65 column slices), S2 -> K2 -> A = K2+eps*I,
#      Newton-Schulz inverse (34 iters, first 20 in bf16, dual X/Z iterate,
#      2-head block-diagonal batching).
#   C: S3 m-major (block-diag landmark lhsT), exp, PE-transpose chunks,
#      E3V accumulation, normalize by r3 -> K3V.
#   GT/GW: G^T = K3V^T Z, GW_h = G_h @ Wp_h (m x 768, pair-stacked).
#   D: per 128-row chunk: S1 row-major, exp, row-normalize, PE-transpose,
#      fused (K1 @ GW) projection, +bias/2, DMA out.

import numpy as np

B, N, D = 4, 4096, 768
H, M, HD = 12, 64, 64
HG = 6          # heads per core
PAIRS = 3       # head pairs per core
P = 128
NBLK = 256      # phase A/C n-block
NB = N // NBLK  # 8
NCH = N // P    # 32
SCALE = 0.125   # hd^-0.5
EPS = 1e-6
NS_ITERS = 34
NS_BF16 = 20
LSTRIDE = 65    # landmark stride: linspace(0,4095,64) == 65*arange(64)

_PROGRAM = None
DEBUG_PHASE = 0  # 0=full, 1=A only, 2=A+B, 3=A+B+NS, 4=+C, 5=+GT/GW


def _build_program():
    import concourse.bass as bass
    import concourse.mybir as mybir
    import concourse.tile as tile
    from concourse import bacc
    from concourse.masks import make_identity
    from contextlib import ExitStack

    DT = mybir.dt.float32
    BF = mybir.dt.bfloat16
    F32R = mybir.dt.float32r
    EXP = mybir.ActivationFunctionType.Exp
    IDENT = mybir.ActivationFunctionType.Identity
    AX = mybir.AxisListType.X

    def r32(ap):
        return ap.bitcast(F32R)

    nc = bacc.Bacc(trn_type="TRN2", target_bir_lowering=False, debug=False)

    x_d = nc.dram_tensor("x", [N, D], F32R, kind="ExternalInput")
    wqk_d = nc.dram_tensor("wqk", [D, 2 * HG * HD], F32R, kind="ExternalInput")
    bqk_d = nc.dram_tensor("bqk", [2 * HG * HD], DT, kind="ExternalInput")
    wv_d = nc.dram_tensor("wv", [D, HG * HD], F32R, kind="ExternalInput")
    bv_d = nc.dram_tensor("bv", [HG * HD], DT, kind="ExternalInput")
    wp_d = nc.dram_tensor("wp", [HG * HD, D], F32R, kind="ExternalInput")
    bph_d = nc.dram_tensor("bph", [D], DT, kind="ExternalInput")
    xl_d = nc.dram_tensor("xl", [M, D], DT, kind="ExternalInput")
    wqk32_d = nc.dram_tensor("wqk32", [D, 2 * HG * HD], DT, kind="ExternalInput")
    out_d = nc.dram_tensor("out", [N, D], DT, kind="ExternalOutput")

    with tile.TileContext(nc) as tc, ExitStack() as ctx:
        singles = ctx.enter_context(tc.tile_pool(name="singles", bufs=1))
        res = ctx.enter_context(tc.tile_pool(name="res", bufs=1))
        dram = ctx.enter_context(tc.tile_pool(name="dram", bufs=1, space="DRAM"))

        ident = singles.tile([P, P], DT)
        make_identity(nc, ident)
        identr = singles.tile([P, P], F32R)
        nc.vector.tensor_copy(identr, ident)
        twoI = singles.tile([P, P], DT)
        nc.vector.tensor_scalar_mul(twoI, ident, 2.0)
        epsI = singles.tile([P, M], DT)  # [eps*I64 ; eps*I64] stacked
        nc.vector.tensor_scalar_mul(epsI[0:M, :], ident[0:M, 0:M], EPS)
        nc.vector.tensor_scalar_mul(epsI[M:P, :], ident[M:P, M:P], EPS)

        biasqk = singles.tile([P, 2 * PAIRS], DT)
        nc.sync.dma_start(out=biasqk, in_=bqk_d.ap().rearrange("(c p) -> p c", p=P))
        bv_bc = singles.tile([P, HG * HD], DT)
        bv_ap = bv_d.ap()
        nc.sync.dma_start(
            out=bv_bc,
            in_=bass.AP(tensor=bv_ap.tensor, offset=bv_ap.offset,
                        ap=[[0, P], [1, HG * HD]]),
        )
        bp_bc = singles.tile([P, D], DT)
        bp_ap = bph_d.ap()
        nc.sync.dma_start(
            out=bp_bc,
            in_=bass.AP(tensor=bp_ap.tensor, offset=bp_ap.offset,
                        ap=[[0, P], [1, D]]),
        )
        wqk_sb = singles.tile([P, 6, 2 * HG * HD], F32R)
        nc.sync.dma_start(out=wqk_sb, in_=wqk_d.ap().rearrange("(c p) f -> p c f", p=P))
        wv_sb = singles.tile([P, 6, HG * HD], F32R)
        nc.sync.dma_start(out=wv_sb, in_=wv_d.ap().rearrange("(c p) f -> p c f", p=P))
        wp_sb = singles.tile([M, HG, D], F32R)
        nc.sync.dma_start(out=wp_sb, in_=wp_d.ap().rearrange("(h p) f -> p h f", p=M))

        QT = res.tile([P, PAIRS, N], F32R)   # partitions = pair-stacked head dims
        KT = res.tile([P, PAIRS, N], F32R)
        gw_sb = res.tile([P, PAIRS, D], F32R)
        v_dram = dram.tile([N, HG * HD], F32R)

        # ---------------- Phase A: qkv projection ----------------
        with tc.tile_pool(name="pa", bufs=3) as pa, \
             tc.tile_pool(name="pa_vt", bufs=3) as pavt, \
             tc.tile_pool(name="pap_t", bufs=3, space="PSUM") as papt, \
             tc.tile_pool(name="pap_qk", bufs=2, space="PSUM") as papqk, \
             tc.tile_pool(name="pap_v", bufs=2, space="PSUM") as papv:
            for nb in range(NB):
                nsl = slice(nb * NBLK, (nb + 1) * NBLK)
                xin = pa.tile([P, NBLK // P, D], F32R, tag="xin")
                nc.sync.dma_start(
                    out=xin, in_=x_d.ap()[nsl, :].rearrange("(c p) d -> p c d", p=P))
                xt = pa.tile([P, 6, NBLK], F32R, tag="xt")
                for c4 in range(NBLK // P):
                    for dc in range(6):
                        tp = papt.tile([P, P], F32R, tag="tp")
                        nc.tensor.transpose(
                            tp, xin[:, c4, dc * P:(dc + 1) * P], identr)
                        nc.any.tensor_copy(
                            out=xt[:, dc, c4 * P:(c4 + 1) * P],
                            in_=tp.bitcast(DT))
                if DEBUG_PHASE == 7 and nb == 0:
                    with tc.tile_pool(name="dbg7", bufs=1) as dbg7:
                        t7 = dbg7.tile([P, D], DT)
                        for dc in range(3):
                            nc.vector.tensor_copy(
                                t7[:, dc * NBLK:(dc + 1) * NBLK],
                                xt[:, dc, :].bitcast(DT))
                        nc.sync.dma_start(out=out_d.ap()[0:P, :], in_=t7)
                        t8 = dbg7.tile([P, D], DT)
                        nc.vector.tensor_copy(t8[:, 0:D], xin[:, 0, :].bitcast(DT))
                        nc.sync.dma_start(out=out_d.ap()[P:2 * P, :], in_=t8)
                for oc in range(6):
                    ps = papqk.tile([P, NBLK], DT, tag="qk")
                    for dc in range(6):
                        nc.tensor.matmul(
                            ps, wqk_sb[:, dc, oc * P:(oc + 1) * P],
                            xt[:, dc, :],
                            start=(dc == 0), stop=(dc == 5))
                    dest = QT if oc < 3 else KT
                    col = oc if oc < 3 else oc - 3
                    nc.scalar.activation(
                        out=dest[:, col, nsl], in_=ps, func=IDENT,
                        bias=biasqk[:, oc:oc + 1], scale=1.0)
                for c4 in range(NBLK // P):
                    psv = papv.tile([P, HG * HD], DT, tag="v")
                    for dc in range(6):
                        nc.tensor.matmul(
                            psv, xt[:, dc, c4 * P:(c4 + 1) * P],
                            wv_sb[:, dc, :],
                            start=(dc == 0), stop=(dc == 5))
                    vt = pavt.tile([P, HG * HD], F32R, tag="vt")
                    nc.vector.tensor_add(vt, psv, bv_bc)
                    nc.sync.dma_start(
                        out=v_dram[(nb * (NBLK // P) + c4) * P:(nb * (NBLK // P) + c4 + 1) * P, :],
                        in_=vt)

        if DEBUG_PHASE == 1:
            with tc.tile_pool(name="dbg", bufs=2) as dbg:
                for ncx in range(NCH):
                    t = dbg.tile([P, D], DT, tag="dbg")
                    nc.vector.memset(t, 0.0)
                    nc.vector.tensor_copy(t[:, 0:128], QT[:, 0, ncx * P:(ncx + 1) * P].bitcast(DT))
                    nc.vector.tensor_copy(t[:, 128:256], KT[:, 0, ncx * P:(ncx + 1) * P].bitcast(DT))
                    nc.sync.dma_start(out=out_d.ap()[ncx * P:(ncx + 1) * P, :], in_=t)

        # ---------------- Phase B: landmarks, A matrices ----------------
        RUN = lambda k: DEBUG_PHASE == 0 or DEBUG_PHASE >= k
        pb = ctx.enter_context(tc.tile_pool(name="pb", bufs=1))
        with tc.tile_pool(name="pbp", bufs=2, space="PSUM") as pbp, \
             tc.tile_pool(name="pbt", bufs=1) as pbt:
            # fp32 landmark path: A = K2 + eps*I must be fp32-exact because
            # cond(A) ~ 2e4 amplifies f32r rounding in the inverse.
            qlt_blk = pb.tile([P, PAIRS, P], F32R)  # block-diag Q_l^T per pair
            klt_blk = pb.tile([P, PAIRS, P], F32R)  # block-diag K_l^T per pair
            nc.vector.memset(qlt_blk.bitcast(DT), 0.0)
            nc.vector.memset(klt_blk.bitcast(DT), 0.0)
            xl = pbt.tile([M, D], DT)
            nc.sync.dma_start(out=xl, in_=xl_d.ap())
            wqk32 = pbt.tile([P, 6, 2 * HG * HD], DT)
            nc.sync.dma_start(
                out=wqk32,
                in_=wqk32_d.ap().rearrange("(c p) f -> p c f", p=P))
            xlt = pbt.tile([P, 6, M], DT)
            for dc in range(6 if RUN(2) else 0):
                tx = pbp.tile([P, M], DT, tag="xlt")
                nc.tensor.transpose(tx, xl[:, dc * P:(dc + 1) * P],
                                    ident[0:M, 0:M])
                nc.any.tensor_copy(out=xlt[:, dc, :], in_=tx)
            qkl32 = pbt.tile([P, 6, M], DT)  # oc 0-2: Q_l^T pairs, 3-5: K_l^T
            for oc in range(6 if RUN(2) else 0):
                pql = pbp.tile([P, M], DT, tag="pql")
                for dc in range(6):
                    nc.tensor.matmul(pql, wqk32[:, dc, oc * P:(oc + 1) * P],
                                     xlt[:, dc, :],
                                     start=(dc == 0), stop=(dc == 5))
                nc.scalar.activation(out=qkl32[:, oc, :], in_=pql, func=IDENT,
                                     bias=biasqk[:, oc:oc + 1], scale=1.0)
            qlt_blk32 = pbt.tile([P, PAIRS, P], DT)
            nc.vector.memset(qlt_blk32, 0.0)
            for pr in range(PAIRS if RUN(2) else 0):
                nc.any.tensor_copy(out=klt_blk[0:M, pr, 0:M],
                                   in_=qkl32[0:M, 3 + pr, :])
                nc.any.tensor_copy(out=klt_blk[M:P, pr, M:P],
                                   in_=qkl32[M:P, 3 + pr, :])
                nc.any.tensor_copy(out=qlt_blk[0:M, pr, 0:M],
                                   in_=qkl32[0:M, pr, :])
                nc.any.tensor_copy(out=qlt_blk[M:P, pr, M:P],
                                   in_=qkl32[M:P, pr, :])
                nc.any.tensor_copy(out=qlt_blk32[0:M, pr, 0:M],
                                   in_=qkl32[0:M, pr, :])
                nc.any.tensor_copy(out=qlt_blk32[M:P, pr, M:P],
                                   in_=qkl32[M:P, pr, :])
            A_st = pb.tile([P, PAIRS, M], DT)     # pair-stacked A = K2 + eps*I
            r2 = pb.tile([P, PAIRS], DT)
            for pr in range(PAIRS if RUN(2) else 0):
                ps2 = pbp.tile([P, M], DT, tag="s2")
                nc.tensor.matmul(ps2, qlt_blk32[:, pr, :], qkl32[:, 3 + pr, :],
                                 start=True, stop=True)
                e2 = pb.tile([P, M], DT, tag=f"e2_{pr}")
                nc.scalar.activation(out=e2, in_=ps2, func=EXP, scale=SCALE,
                                     accum_out=r2[:, pr:pr + 1])
                r2c = pb.tile([P, 1], DT, tag=f"r2c_{pr}")
                nc.vector.reciprocal(r2c, r2[:, pr:pr + 1])
                nc.vector.tensor_scalar_mul(A_st[:, pr, :], e2, r2c)
                nc.vector.tensor_add(A_st[:, pr, :], A_st[:, pr, :], epsI)

        if DEBUG_PHASE == 2:
            with tc.tile_pool(name="dbg", bufs=2) as dbg:
                t = dbg.tile([P, D], DT, tag="dbg")
                nc.vector.memset(t, 0.0)
                nc.vector.tensor_copy(t[:, 0:3 * M], A_st.bitcast(DT) if A_st.dtype != DT else A_st)
                nc.sync.dma_start(out=out_d.ap()[0:P, :], in_=t)

        # ---------------- Newton-Schulz inverse (dual iterate) ----------------
        ns_x = []
        ns_z = []
        with tc.tile_pool(name="nsp", bufs=2, space="PSUM") as nsp:
            for pr in range(PAIRS if RUN(3) else 0):
                Ablk = pb.tile([P, P], DT, tag=f"ablk{pr}")
                nc.vector.memset(Ablk, 0.0)
                nc.any.tensor_copy(out=Ablk[0:M, 0:M], in_=A_st[0:M, pr, :])
                nc.any.tensor_copy(out=Ablk[M:P, M:P], in_=A_st[M:P, pr, :])
                tb = nsp.tile([P, P], DT, tag="ns")
                nc.tensor.transpose(tb, Ablk, ident)  # A^T blockdiag
                b_f = pb.tile([P, P], DT, tag=f"bf{pr}")
                nc.any.tensor_copy(out=b_f, in_=tb)
                b_bf = pb.tile([P, P], BF, tag=f"bbf{pr}")
                nc.any.tensor_copy(out=b_bf, in_=tb)
                x_bf = pb.tile([P, P], BF, tag=f"xbf{pr}")
                nc.any.tensor_copy(out=x_bf, in_=tb)        # X0 = A^T (a0=1)
                z_bf = pb.tile([P, P], BF, tag=f"zbf{pr}")
                nc.any.tensor_copy(out=z_bf, in_=Ablk)      # Z0 = X0^T = A
                x_f = pb.tile([P, P], DT, tag=f"xf{pr}")
                z_f = pb.tile([P, P], DT, tag=f"zf{pr}")
                for it in range(NS_ITERS):
                    bf_now = it < NS_BF16
                    bf_next = (it + 1) < NS_BF16
                    cx = x_bf if bf_now else x_f
                    cz = z_bf if bf_now else z_f
                    cb = b_bf if bf_now else b_f
                    t1 = nsp.tile([P, P], DT, tag="ns")
                    nc.tensor.matmul(t1, cb, cx, start=True, stop=True)  # A@X
                    u = pb.tile([P, P], BF if bf_now else DT,
                                tag=f"u{pr}_{it % 2}_{int(bf_now)}")
                    nc.vector.tensor_sub(u, twoI, t1)        # U = 2I - A X
                    xn = nsp.tile([P, P], DT, tag="ns")
                    nc.tensor.matmul(xn, cz, u, start=True, stop=True)   # X@U
                    zn = nsp.tile([P, P], DT, tag="ns")
                    nc.tensor.matmul(zn, u, cz, start=True, stop=True)   # U^T@Z
                    if bf_next:
                        nc.any.tensor_copy(out=x_bf, in_=xn)
                        nc.any.tensor_copy(out=z_bf, in_=zn)
                    else:
                        nc.any.tensor_copy(out=x_f, in_=xn)
                        nc.any.tensor_copy(out=z_f, in_=zn)
                ns_x.append(x_f)
                ns_z.append(z_f)

        if DEBUG_PHASE == 3:
            with tc.tile_pool(name="dbg", bufs=2) as dbg:
                t = dbg.tile([P, D], DT, tag="dbg")
                nc.vector.memset(t, 0.0)
                for pr in range(PAIRS):
                    nc.vector.tensor_copy(t[:, pr * P:(pr + 1) * P], ns_z[pr])
                nc.sync.dma_start(out=out_d.ap()[0:P, :], in_=t)

        # ---------------- Phase C: S3, E3V, K3V ----------------
        k3v_st = pb.tile([P, PAIRS, M], DT)
        with tc.tile_pool(name="pc", bufs=3) as pc, \
             tc.tile_pool(name="pcp_s3", bufs=2, space="PSUM") as pcps3, \
             tc.tile_pool(name="pcp_t", bufs=2, space="PSUM") as pcpt, \
             tc.tile_pool(name="pcp_acc", bufs=1, space="PSUM") as pcpacc:
            r3acc = pb.tile([P, PAIRS, NB], DT)
            k3vps = pcpacc.tile([P, PAIRS, P], DT)  # one bank, 3 accum regions
            for nb in range(NB if RUN(4) else 0):
                nsl = slice(nb * NBLK, (nb + 1) * NBLK)
                vblk = pc.tile([P, NBLK // P, HG * HD], F32R, tag="vblk")
                nc.sync.dma_start(
                    out=vblk,
                    in_=v_dram[nsl, :].rearrange("(c p) f -> p c f", p=P))
                for pr in range(PAIRS):
                    s3 = pcps3.tile([P, NBLK], DT, tag="s3")
                    nc.tensor.matmul(s3, qlt_blk[:, pr, :],
                                     KT[:, pr, nsl], start=True, stop=True)
                    e3 = pc.tile([P, NBLK], F32R, tag="e3")
                    nc.scalar.activation(out=e3, in_=s3, func=EXP, scale=SCALE,
                                         accum_out=r3acc[:, pr, nb:nb + 1])
                    for c4 in range(NBLK // P):
                        tp = pcpt.tile([P, P], F32R, tag="e3t")
                        nc.tensor.transpose(
                            tp, e3[:, c4 * P:(c4 + 1) * P], identr)
                        e3t = pc.tile([P, P], F32R, tag="e3ts")
                        nc.any.tensor_copy(out=e3t, in_=tp.bitcast(DT))
                        first = (nb == 0 and pr == 0 and c4 == 0)
                        last = (nb == NB - 1 and c4 == NBLK // P - 1)
                        nc.tensor.matmul(
                            k3vps[:, pr, :], e3t,
                            vblk[:, c4, pr * P:(pr + 1) * P],
                            start=first, stop=last, skip_group_check=True)
            for pr in range(PAIRS if RUN(4) else 0):
                r3 = pb.tile([P, 1], DT, tag=f"r3_{pr}")
                nc.vector.reduce_sum(r3, r3acc[:, pr, :], axis=AX)
                r3c = pb.tile([P, 1], DT, tag=f"r3c_{pr}")
                nc.vector.reciprocal(r3c, r3)
                for h2 in range(2):
                    sl = slice(h2 * M, (h2 + 1) * M)
                    nc.vector.tensor_scalar_mul(
                        k3v_st[sl, pr, :], k3vps[sl, pr, h2 * M:(h2 + 1) * M],
                        r3c[sl])

        if DEBUG_PHASE == 4:
            with tc.tile_pool(name="dbg", bufs=2) as dbg:
                t = dbg.tile([P, D], DT, tag="dbg")
                nc.vector.memset(t, 0.0)
                nc.vector.tensor_copy(t[:, 0:3 * M], k3v_st)
                nc.sync.dma_start(out=out_d.ap()[0:P, :], in_=t)

        # ---------------- GT / GW ----------------
        with tc.tile_pool(name="pg", bufs=1) as pg, \
             tc.tile_pool(name="pgp", bufs=2, space="PSUM") as pgp:
            for pr in range(PAIRS if RUN(5) else 0):
                gtp = pgp.tile([M, P], DT, tag="gt")
                nc.tensor.matmul(gtp, k3v_st[:, pr, :], ns_z[pr],
                                 start=True, stop=True)  # [GT_h1 | GT_h2]
                gt = pg.tile([M, P], F32R, tag=f"gt{pr}")
                nc.any.tensor_copy(out=gt, in_=gtp)
                for h2 in range(2):
                    lt = gt[:, h2 * M:(h2 + 1) * M]
                    rh = wp_sb[:, 2 * pr + h2, :]
                    g1 = pgp.tile([M, 512], DT, tag="gw1")
                    nc.tensor.matmul(g1, lt, rh[:, 0:512],
                                     start=True, stop=True)
                    g2 = pgp.tile([M, 256], DT, tag="gw2")
                    nc.tensor.matmul(g2, lt, rh[:, 512:768],
                                     start=True, stop=True)
                    nc.any.tensor_copy(
                        out=gw_sb[h2 * M:(h2 + 1) * M, pr, 0:512], in_=g1)
                    nc.any.tensor_copy(
                        out=gw_sb[h2 * M:(h2 + 1) * M, pr, 512:768], in_=g2)

        if DEBUG_PHASE == 5:
            with tc.tile_pool(name="dbg", bufs=2) as dbg:
                t = dbg.tile([P, D], DT, tag="dbg")
                nc.vector.tensor_copy(t, gw_sb[:, 0, :].bitcast(DT))
                nc.sync.dma_start(out=out_d.ap()[0:P, :], in_=t)

        # ---------------- Phase D: S1, K1, fused projection ----------------
        with tc.tile_pool(name="pd", bufs=3) as pd, \
             tc.tile_pool(name="pd_k1", bufs=2) as pdk1, \
             tc.tile_pool(name="pdp_s1", bufs=2, space="PSUM") as pdps1, \
             tc.tile_pool(name="pdp_t", bufs=2, space="PSUM") as pdpt, \
             tc.tile_pool(name="pdp_o", bufs=2, space="PSUM") as pdpo:
            ncount = NCH if DEBUG_PHASE != 8 else 1
            for ncx in range(ncount if RUN(6) else 0):
                nsl = slice(ncx * P, (ncx + 1) * P)
                s1 = pdps1.tile([P, HG * M], DT, tag="s1")
                for pr in range(PAIRS):
                    nc.tensor.matmul(
                        s1[:, pr * P:(pr + 1) * P],
                        QT[:, pr, nsl], klt_blk[:, pr, :],
                        start=(pr == 0), stop=(pr == PAIRS - 1),
                        skip_group_check=True)
                e1 = pd.tile([P, HG * M], DT, tag="e1")
                nc.scalar.activation(out=e1, in_=s1, func=EXP, scale=SCALE)
                r1 = pd.tile([P, HG], DT, tag="r1")
                nc.vector.reduce_sum(
                    r1, e1.rearrange("p (h m) -> p h m", h=HG), axis=AX)
                rc = pd.tile([P, HG], DT, tag="rc")
                nc.vector.reciprocal(rc, r1)
                e1n = pd.tile([P, HG * M], F32R, tag="e1n")
                for h6 in range(HG):
                    nc.vector.tensor_scalar_mul(
                        e1n[:, h6 * M:(h6 + 1) * M],
                        e1[:, h6 * M:(h6 + 1) * M], rc[:, h6:h6 + 1])
                k1ts = []
                for kc in range(PAIRS):
                    tp = pdpt.tile([P, P], F32R, tag="k1tp")
                    nc.tensor.transpose(
                        tp, e1n[:, kc * P:(kc + 1) * P], identr)
                    k1t = pdk1.tile([P, P], F32R, tag=f"k1t{kc}")
                    nc.any.tensor_copy(out=k1t, in_=tp.bitcast(DT))
                    k1ts.append(k1t)
                po1 = pdpo.tile([P, 512], DT, tag="po1")
                po2 = pdpo.tile([P, 256], DT, tag="po2")
                for kc in range(PAIRS):
                    nc.tensor.matmul(po1, k1ts[kc], gw_sb[:, kc, 0:512],
                                     start=(kc == 0), stop=(kc == PAIRS - 1))
                    nc.tensor.matmul(po2, k1ts[kc], gw_sb[:, kc, 512:768],
                                     start=(kc == 0), stop=(kc == PAIRS - 1))
                ob = pd.tile([P, D], DT, tag="ob")
                nc.vector.tensor_add(ob[:, 0:512], po1, bp_bc[:, 0:512])
                nc.vector.tensor_add(ob[:, 512:768], po2, bp_bc[:, 512:768])
                nc.sync.dma_start(out=out_d.ap()[nsl, :], in_=ob)

    nc.compile()
    return nc


def get_program():
    global _PROGRAM
    if _PROGRAM is None:
        _PROGRAM = _build_program()
    return _PROGRAM


def shard_inputs(x, w_qkv, b_qkv, w_proj, b_proj):
    """Returns one in_map per core (core = 2*b + head_group)."""
    x = np.ascontiguousarray(x, dtype=np.float32)
    w_qkv = np.asarray(w_qkv, dtype=np.float32)
    b_qkv = np.asarray(b_qkv, dtype=np.float32)
    w_proj = np.asarray(w_proj, dtype=np.float32)
    b_proj = np.asarray(b_proj, dtype=np.float32)
    wq, wk, wv = w_qkv[:, 0:D], w_qkv[:, D:2 * D], w_qkv[:, 2 * D:3 * D]
    bq, bk, bv = b_qkv[0:D], b_qkv[D:2 * D], b_qkv[2 * D:3 * D]
    in_maps = []
    for core in range(8):
        b, hg = core // 2, core % 2
        cs = slice(hg * HG * HD, (hg + 1) * HG * HD)
        wqk_c = np.ascontiguousarray(
            np.concatenate([wq[:, cs], wk[:, cs]], axis=1))
        in_maps.append({
            "x": np.ascontiguousarray(x[b]),
            "wqk": wqk_c,
            "xl": np.ascontiguousarray(x[b][0:4096:65, :]),
            "wqk32": wqk_c.copy(),
            "bqk": np.ascontiguousarray(
                np.concatenate([bq[cs], bk[cs]])),
            "wv": np.ascontiguousarray(wv[:, cs]),
            "bv": np.ascontiguousarray(bv[cs]),
            "wp": np.ascontiguousarray(w_proj[cs.start:cs.stop, :]),
            "bph": np.ascontiguousarray(0.5 * b_proj),
        })
    return in_maps


def run_cores(in_maps, trace=False, **kw):
    from concourse import bass_utils
    nc = get_program()
    return bass_utils.run_bass_kernel_spmd(
        nc, in_maps, core_ids=list(range(8)), trace=trace, **kw)


def unshard_output(results):
    out = np.empty((B, N, D), dtype=np.float32)
    for b in range(B):
        out[b] = results[2 * b]["out"] + results[2 * b + 1]["out"]
    return out


def kernel(x, w_qkv, b_qkv, w_proj, b_proj):
    in_maps = shard_inputs(x, w_qkv, b_qkv, w_proj, b_proj)
    res = run_cores(in_maps)
    return unshard_output(res.results)


# revision 21
# speedup vs baseline: 1.2765x; 1.2763x over previous
# Nystrom attention TRN2 kernel (B=4, N=4096, D=768, H=12, m=64 landmarks).
#
# Sharding: 8 cores; core c handles batch b = c//2 and a 6-head group
# hg = c%2 (heads 6*hg .. 6*hg+5, organized as 3 adjacent pairs).
# Each core computes its heads' full contribution through w_proj plus half
# the proj bias; the host unshards by summing the two partials per batch.
#
# Per-core pipeline (all matmuls on PE, fp32 data, fp32r perf mode where the
# moving free dim is large):
#   A: stream x in 512-row blocks, PE-transpose to get D-major X^T, then
#      QT/KT head-major (pair-stacked on partitions) resident in SBUF and
#      V row-major streamed to a DRAM scratch buffer.
#   B: landmark gathers (stride-65 column slices), S2 -> K2 -> A = K2+eps*I,
#      Newton-Schulz inverse (34 iters, first 20 in bf16, dual X/Z iterate,
#      2-head block-diagonal batching).
#   C: S3 m-major (block-diag landmark lhsT), exp, PE-transpose chunks,
#      E3V accumulation, normalize by r3 -> K3V.
#   GT/GW: G^T = K3V^T Z, GW_h = G_h @ Wp_h (m x 768, pair-stacked).
#   D: per 128-row chunk: S1 row-major, exp, row-normalize, PE-transpose,
#      fused (K1 @ GW) projection, +bias/2, DMA out.

import numpy as np

B, N, D = 4, 4096, 768
H, M, HD = 12, 64, 64
HG = 6          # heads per core
PAIRS = 3       # head pairs per core
P = 128
NBLK = 256      # phase A/C n-block
NB = N // NBLK  # 8
NCH = N // P    # 32
SCALE = 0.125   # hd^-0.5
EPS = 1e-6
NS_ITERS = 32
NS_BF16 = 20
LSTRIDE = 65    # landmark stride: linspace(0,4095,64) == 65*arange(64)

_PROGRAM = None
DEBUG_PHASE = 0  # 0=full, 1=A only, 2=A+B, 3=A+B+NS, 4=+C, 5=+GT/GW


def _build_program():
    import concourse.bass as bass
    import concourse.mybir as mybir
    import concourse.tile as tile
    from concourse import bacc
    from concourse.masks import make_identity
    from contextlib import ExitStack

    DT = mybir.dt.float32
    BF = mybir.dt.bfloat16
    F32R = mybir.dt.float32r
    EXP = mybir.ActivationFunctionType.Exp
    IDENT = mybir.ActivationFunctionType.Identity
    AX = mybir.AxisListType.X

    def r32(ap):
        return ap.bitcast(F32R)

    nc = bacc.Bacc(trn_type="TRN2", target_bir_lowering=False, debug=False)

    x_d = nc.dram_tensor("x", [N, D], F32R, kind="ExternalInput")
    wqk_d = nc.dram_tensor("wqk", [D, 2 * HG * HD], F32R, kind="ExternalInput")
    bqk_d = nc.dram_tensor("bqk", [2 * HG * HD], DT, kind="ExternalInput")
    wv_d = nc.dram_tensor("wv", [D, HG * HD], F32R, kind="ExternalInput")
    bv_d = nc.dram_tensor("bv", [HG * HD], DT, kind="ExternalInput")
    wp_d = nc.dram_tensor("wp", [HG * HD, D], F32R, kind="ExternalInput")
    bph_d = nc.dram_tensor("bph", [D], DT, kind="ExternalInput")
    xl_d = nc.dram_tensor("xl", [M, D], DT, kind="ExternalInput")
    wqk32_d = nc.dram_tensor("wqk32", [D, 2 * HG * HD], DT, kind="ExternalInput")
    out_d = nc.dram_tensor("out", [N, D], DT, kind="ExternalOutput")

    with tile.TileContext(nc) as tc, ExitStack() as ctx:
        singles = ctx.enter_context(tc.tile_pool(name="singles", bufs=1))
        res = ctx.enter_context(tc.tile_pool(name="res", bufs=1))
        dram = ctx.enter_context(tc.tile_pool(name="dram", bufs=1, space="DRAM"))

        ident = singles.tile([P, P], DT)
        make_identity(nc, ident)
        identr = singles.tile([P, P], F32R)
        nc.vector.tensor_copy(identr, ident)
        twoI = singles.tile([P, P], DT)
        nc.vector.tensor_scalar_mul(twoI, ident, 2.0)
        epsI = singles.tile([P, M], DT)  # [eps*I64 ; eps*I64] stacked
        nc.vector.tensor_scalar_mul(epsI[0:M, :], ident[0:M, 0:M], EPS)
        nc.vector.tensor_scalar_mul(epsI[M:P, :], ident[M:P, M:P], EPS)

        biasqk = singles.tile([P, 2 * PAIRS], DT)
        nc.sync.dma_start(out=biasqk, in_=bqk_d.ap().rearrange("(c p) -> p c", p=P))
        bv_bc = singles.tile([P, HG * HD], DT)
        bv_ap = bv_d.ap()
        nc.sync.dma_start(
            out=bv_bc,
            in_=bass.AP(tensor=bv_ap.tensor, offset=bv_ap.offset,
                        ap=[[0, P], [1, HG * HD]]),
        )
        bp_bc = singles.tile([P, D], DT)
        bp_ap = bph_d.ap()
        nc.sync.dma_start(
            out=bp_bc,
            in_=bass.AP(tensor=bp_ap.tensor, offset=bp_ap.offset,
                        ap=[[0, P], [1, D]]),
        )
        wqk_sb = singles.tile([P, 6, 2 * HG * HD], F32R)
        nc.sync.dma_start(out=wqk_sb, in_=wqk_d.ap().rearrange("(c p) f -> p c f", p=P))
        wv_sb = singles.tile([P, 6, HG * HD], F32R)
        nc.sync.dma_start(out=wv_sb, in_=wv_d.ap().rearrange("(c p) f -> p c f", p=P))
        wp_sb = singles.tile([M, HG, D], F32R)
        nc.sync.dma_start(out=wp_sb, in_=wp_d.ap().rearrange("(h p) f -> p h f", p=M))

        QT = res.tile([P, PAIRS, N], F32R)   # partitions = pair-stacked head dims
        KT = res.tile([P, PAIRS, N], F32R)
        gw_sb = res.tile([P, PAIRS, D], F32R)
        v_dram = dram.tile([N, HG * HD], F32R)

        # ---------------- Phase A: qkv projection ----------------
        with tc.tile_pool(name="pa", bufs=3) as pa, \
             tc.tile_pool(name="pa_vt", bufs=3) as pavt, \
             tc.tile_pool(name="pap_t", bufs=3, space="PSUM") as papt, \
             tc.tile_pool(name="pap_qk", bufs=2, space="PSUM") as papqk, \
             tc.tile_pool(name="pap_v", bufs=2, space="PSUM") as papv:
            for nb in range(NB):
                nsl = slice(nb * NBLK, (nb + 1) * NBLK)
                xin = pa.tile([P, NBLK // P, D], F32R, tag="xin")
                nc.sync.dma_start(
                    out=xin, in_=x_d.ap()[nsl, :].rearrange("(c p) d -> p c d", p=P))
                xt = pa.tile([P, 6, NBLK], F32R, tag="xt")
                for c4 in range(NBLK // P):
                    for dc in range(6):
                        tp = papt.tile([P, P], F32R, tag="tp")
                        nc.tensor.transpose(
                            tp, xin[:, c4, dc * P:(dc + 1) * P], identr)
                        nc.any.tensor_copy(
                            out=xt[:, dc, c4 * P:(c4 + 1) * P],
                            in_=tp.bitcast(DT))
                if DEBUG_PHASE == 7 and nb == 0:
                    with tc.tile_pool(name="dbg7", bufs=1) as dbg7:
                        t7 = dbg7.tile([P, D], DT)
                        for dc in range(3):
                            nc.vector.tensor_copy(
                                t7[:, dc * NBLK:(dc + 1) * NBLK],
                                xt[:, dc, :].bitcast(DT))
                        nc.sync.dma_start(out=out_d.ap()[0:P, :], in_=t7)
                        t8 = dbg7.tile([P, D], DT)
                        nc.vector.tensor_copy(t8[:, 0:D], xin[:, 0, :].bitcast(DT))
                        nc.sync.dma_start(out=out_d.ap()[P:2 * P, :], in_=t8)
                for oc in range(6):
                    ps = papqk.tile([P, NBLK], DT, tag="qk")
                    for dc in range(6):
                        nc.tensor.matmul(
                            ps, wqk_sb[:, dc, oc * P:(oc + 1) * P],
                            xt[:, dc, :],
                            start=(dc == 0), stop=(dc == 5))
                    dest = QT if oc < 3 else KT
                    col = oc if oc < 3 else oc - 3
                    nc.scalar.activation(
                        out=dest[:, col, nsl], in_=ps, func=IDENT,
                        bias=biasqk[:, oc:oc + 1], scale=1.0)
                for c4 in range(NBLK // P):
                    psv = papv.tile([P, HG * HD], DT, tag="v")
                    for dc in range(6):
                        nc.tensor.matmul(
                            psv, xt[:, dc, c4 * P:(c4 + 1) * P],
                            wv_sb[:, dc, :],
                            start=(dc == 0), stop=(dc == 5))
                    vt = pavt.tile([P, HG * HD], F32R, tag="vt")
                    nc.vector.tensor_add(vt, psv, bv_bc)
                    nc.sync.dma_start(
                        out=v_dram[(nb * (NBLK // P) + c4) * P:(nb * (NBLK // P) + c4 + 1) * P, :],
                        in_=vt)

        if DEBUG_PHASE == 1:
            with tc.tile_pool(name="dbg", bufs=2) as dbg:
                for ncx in range(NCH):
                    t = dbg.tile([P, D], DT, tag="dbg")
                    nc.vector.memset(t, 0.0)
                    nc.vector.tensor_copy(t[:, 0:128], QT[:, 0, ncx * P:(ncx + 1) * P].bitcast(DT))
                    nc.vector.tensor_copy(t[:, 128:256], KT[:, 0, ncx * P:(ncx + 1) * P].bitcast(DT))
                    nc.sync.dma_start(out=out_d.ap()[ncx * P:(ncx + 1) * P, :], in_=t)

        # ---------------- Phase B: landmarks, A matrices ----------------
        RUN = lambda k: DEBUG_PHASE == 0 or DEBUG_PHASE >= k
        pb = ctx.enter_context(tc.tile_pool(name="pb", bufs=1))
        with tc.tile_pool(name="pbp", bufs=2, space="PSUM") as pbp, \
             tc.tile_pool(name="pbt", bufs=1) as pbt:
            # fp32 landmark path: A = K2 + eps*I must be fp32-exact because
            # cond(A) ~ 2e4 amplifies f32r rounding in the inverse.
            qlt_blk = pb.tile([P, PAIRS, P], F32R)  # block-diag Q_l^T per pair
            klt_blk = pb.tile([P, PAIRS, P], F32R)  # block-diag K_l^T per pair
            nc.vector.memset(qlt_blk.bitcast(DT), 0.0)
            nc.vector.memset(klt_blk.bitcast(DT), 0.0)
            xl = pbt.tile([M, D], DT)
            nc.sync.dma_start(out=xl, in_=xl_d.ap())
            wqk32 = pbt.tile([P, 6, 2 * HG * HD], DT)
            nc.sync.dma_start(
                out=wqk32,
                in_=wqk32_d.ap().rearrange("(c p) f -> p c f", p=P))
            xlt = pbt.tile([P, 6, M], DT)
            for dc in range(6 if RUN(2) else 0):
                tx = pbp.tile([P, M], DT, tag="xlt")
                nc.tensor.transpose(tx, xl[:, dc * P:(dc + 1) * P],
                                    ident[0:M, 0:M])
                nc.any.tensor_copy(out=xlt[:, dc, :], in_=tx)
            qkl32 = pbt.tile([P, 6, M], DT)  # oc 0-2: Q_l^T pairs, 3-5: K_l^T
            for oc in range(6 if RUN(2) else 0):
                pql = pbp.tile([P, M], DT, tag="pql")
                for dc in range(6):
                    nc.tensor.matmul(pql, wqk32[:, dc, oc * P:(oc + 1) * P],
                                     xlt[:, dc, :],
                                     start=(dc == 0), stop=(dc == 5))
                nc.scalar.activation(out=qkl32[:, oc, :], in_=pql, func=IDENT,
                                     bias=biasqk[:, oc:oc + 1], scale=1.0)
            qlt_blk32 = pbt.tile([P, PAIRS, P], DT)
            nc.vector.memset(qlt_blk32, 0.0)
            for pr in range(PAIRS if RUN(2) else 0):
                nc.any.tensor_copy(out=klt_blk[0:M, pr, 0:M],
                                   in_=qkl32[0:M, 3 + pr, :])
                nc.any.tensor_copy(out=klt_blk[M:P, pr, M:P],
                                   in_=qkl32[M:P, 3 + pr, :])
                nc.any.tensor_copy(out=qlt_blk[0:M, pr, 0:M],
                                   in_=qkl32[0:M, pr, :])
                nc.any.tensor_copy(out=qlt_blk[M:P, pr, M:P],
                                   in_=qkl32[M:P, pr, :])
                nc.any.tensor_copy(out=qlt_blk32[0:M, pr, 0:M],
                                   in_=qkl32[0:M, pr, :])
                nc.any.tensor_copy(out=qlt_blk32[M:P, pr, M:P],
                                   in_=qkl32[M:P, pr, :])
            A_st = pb.tile([P, PAIRS, M], DT)     # pair-stacked A = K2 + eps*I
            r2 = pb.tile([P, PAIRS], DT)
            for pr in range(PAIRS if RUN(2) else 0):
                ps2 = pbp.tile([P, M], DT, tag="s2")
                nc.tensor.matmul(ps2, qlt_blk32[:, pr, :], qkl32[:, 3 + pr, :],
                                 start=True, stop=True)
                e2 = pb.tile([P, M], DT, tag=f"e2_{pr}")
                nc.scalar.activation(out=e2, in_=ps2, func=EXP, scale=SCALE,
                                     accum_out=r2[:, pr:pr + 1])
                r2c = pb.tile([P, 1], DT, tag=f"r2c_{pr}")
                nc.vector.reciprocal(r2c, r2[:, pr:pr + 1])
                nc.vector.tensor_scalar_mul(A_st[:, pr, :], e2, r2c)
                nc.vector.tensor_add(A_st[:, pr, :], A_st[:, pr, :], epsI)

        if DEBUG_PHASE == 2:
            with tc.tile_pool(name="dbg", bufs=2) as dbg:
                t = dbg.tile([P, D], DT, tag="dbg")
                nc.vector.memset(t, 0.0)
                nc.vector.tensor_copy(t[:, 0:3 * M], A_st.bitcast(DT) if A_st.dtype != DT else A_st)
                nc.sync.dma_start(out=out_d.ap()[0:P, :], in_=t)

        # ---------------- Phase C: S3, E3V, K3V ----------------
        k3v_st = pb.tile([P, PAIRS, M], DT)
        with tc.tile_pool(name="pc", bufs=3) as pc, \
             tc.tile_pool(name="pcp_s3", bufs=2, space="PSUM") as pcps3, \
             tc.tile_pool(name="pcp_t", bufs=2, space="PSUM") as pcpt, \
             tc.tile_pool(name="pcp_acc", bufs=1, space="PSUM") as pcpacc:
            r3acc = pb.tile([P, PAIRS, NB], DT)
            k3vps = pcpacc.tile([P, PAIRS, P], DT)  # one bank, 3 accum regions
            for nb in range(NB if RUN(4) else 0):
                nsl = slice(nb * NBLK, (nb + 1) * NBLK)
                vblk = pc.tile([P, NBLK // P, HG * HD], F32R, tag="vblk")
                nc.sync.dma_start(
                    out=vblk,
                    in_=v_dram[nsl, :].rearrange("(c p) f -> p c f", p=P))
                for pr in range(PAIRS):
                    s3 = pcps3.tile([P, NBLK], DT, tag="s3")
                    nc.tensor.matmul(s3, qlt_blk[:, pr, :],
                                     KT[:, pr, nsl], start=True, stop=True)
                    e3 = pc.tile([P, NBLK], F32R, tag="e3")
                    nc.scalar.activation(out=e3, in_=s3, func=EXP, scale=SCALE,
                                         accum_out=r3acc[:, pr, nb:nb + 1])
                    for c4 in range(NBLK // P):
                        tp = pcpt.tile([P, P], F32R, tag="e3t")
                        nc.tensor.transpose(
                            tp, e3[:, c4 * P:(c4 + 1) * P], identr)
                        e3t = pc.tile([P, P], F32R, tag="e3ts")
                        nc.any.tensor_copy(out=e3t, in_=tp.bitcast(DT))
                        first = (nb == 0 and pr == 0 and c4 == 0)
                        last = (nb == NB - 1 and c4 == NBLK // P - 1)
                        nc.tensor.matmul(
                            k3vps[:, pr, :], e3t,
                            vblk[:, c4, pr * P:(pr + 1) * P],
                            start=first, stop=last, skip_group_check=True)
            for pr in range(PAIRS if RUN(4) else 0):
                r3 = pb.tile([P, 1], DT, tag=f"r3_{pr}")
                nc.vector.reduce_sum(r3, r3acc[:, pr, :], axis=AX)
                r3c = pb.tile([P, 1], DT, tag=f"r3c_{pr}")
                nc.vector.reciprocal(r3c, r3)
                for h2 in range(2):
                    sl = slice(h2 * M, (h2 + 1) * M)
                    nc.vector.tensor_scalar_mul(
                        k3v_st[sl, pr, :], k3vps[sl, pr, h2 * M:(h2 + 1) * M],
                        r3c[sl])

        # ---------------- Newton-Schulz inverse (dual iterate) ----------------
        ns_x = []
        ns_z = []
        with tc.tile_pool(name="nsp", bufs=2, space="PSUM") as nsp:
            for pr in range(PAIRS if RUN(3) else 0):
                Ablk = pb.tile([P, P], DT, tag=f"ablk{pr}")
                nc.vector.memset(Ablk, 0.0)
                nc.any.tensor_copy(out=Ablk[0:M, 0:M], in_=A_st[0:M, pr, :])
                nc.any.tensor_copy(out=Ablk[M:P, M:P], in_=A_st[M:P, pr, :])
                tb = nsp.tile([P, P], DT, tag=f"ns{pr}")
                nc.tensor.transpose(tb, Ablk, ident)  # A^T blockdiag
                b_f = pb.tile([P, P], DT, tag=f"bf{pr}")
                nc.any.tensor_copy(out=b_f, in_=tb)
                b_bf = pb.tile([P, P], BF, tag=f"bbf{pr}")
                nc.any.tensor_copy(out=b_bf, in_=tb)
                x_bf = pb.tile([P, P], BF, tag=f"xbf{pr}")
                nc.any.tensor_copy(out=x_bf, in_=tb)        # X0 = A^T (a0=1)
                z_bf = pb.tile([P, P], BF, tag=f"zbf{pr}")
                nc.any.tensor_copy(out=z_bf, in_=Ablk)      # Z0 = X0^T = A
                x_f = pb.tile([P, P], DT, tag=f"xf{pr}")
                z_f = pb.tile([P, P], DT, tag=f"zf{pr}")
                for it in range(NS_ITERS):
                    bf_now = it < NS_BF16
                    bf_next = (it + 1) < NS_BF16
                    cx = x_bf if bf_now else x_f
                    cz = z_bf if bf_now else z_f
                    cb = b_bf if bf_now else b_f
                    t1 = nsp.tile([P, P], DT, tag=f"ns{pr}")
                    nc.tensor.matmul(t1, cb, cx, start=True, stop=True)  # A@X
                    u = pb.tile([P, P], BF if bf_now else DT,
                                tag=f"u{pr}_{it % 2}_{int(bf_now)}")
                    nc.vector.tensor_sub(u, twoI, t1)        # U = 2I - A X
                    xn = nsp.tile([P, P], DT, tag=f"ns{pr}")
                    nc.tensor.matmul(xn, cz, u, start=True, stop=True)   # X@U
                    zn = nsp.tile([P, P], DT, tag=f"ns{pr}")
                    nc.tensor.matmul(zn, u, cz, start=True, stop=True)   # U^T@Z
                    if bf_next:
                        nc.any.tensor_copy(out=x_bf, in_=xn)
                        nc.any.tensor_copy(out=z_bf, in_=zn)
                    else:
                        nc.any.tensor_copy(out=x_f, in_=xn)
                        nc.any.tensor_copy(out=z_f, in_=zn)
                ns_x.append(x_f)
                ns_z.append(z_f)

        if DEBUG_PHASE == 3:
            with tc.tile_pool(name="dbg", bufs=2) as dbg:
                t = dbg.tile([P, D], DT, tag="dbg")
                nc.vector.memset(t, 0.0)
                for pr in range(PAIRS):
                    nc.vector.tensor_copy(t[:, pr * P:(pr + 1) * P], ns_z[pr])
                nc.sync.dma_start(out=out_d.ap()[0:P, :], in_=t)

        if DEBUG_PHASE == 4:
            with tc.tile_pool(name="dbg", bufs=2) as dbg:
                t = dbg.tile([P, D], DT, tag="dbg")
                nc.vector.memset(t, 0.0)
                nc.vector.tensor_copy(t[:, 0:3 * M], k3v_st)
                nc.sync.dma_start(out=out_d.ap()[0:P, :], in_=t)

        # ---------------- GT / GW ----------------
        with tc.tile_pool(name="pg", bufs=1) as pg, \
             tc.tile_pool(name="pgp", bufs=2, space="PSUM") as pgp:
            for pr in range(PAIRS if RUN(5) else 0):
                gtp = pgp.tile([M, P], DT, tag="gt")
                nc.tensor.matmul(gtp, k3v_st[:, pr, :], ns_z[pr],
                                 start=True, stop=True)  # [GT_h1 | GT_h2]
                gt = pg.tile([M, P], F32R, tag=f"gt{pr}")
                nc.any.tensor_copy(out=gt, in_=gtp)
                for h2 in range(2):
                    lt = gt[:, h2 * M:(h2 + 1) * M]
                    rh = wp_sb[:, 2 * pr + h2, :]
                    g1 = pgp.tile([M, 512], DT, tag="gw1")
                    nc.tensor.matmul(g1, lt, rh[:, 0:512],
                                     start=True, stop=True)
                    g2 = pgp.tile([M, 256], DT, tag="gw2")
                    nc.tensor.matmul(g2, lt, rh[:, 512:768],
                                     start=True, stop=True)
                    nc.any.tensor_copy(
                        out=gw_sb[h2 * M:(h2 + 1) * M, pr, 0:512], in_=g1)
                    nc.any.tensor_copy(
                        out=gw_sb[h2 * M:(h2 + 1) * M, pr, 512:768], in_=g2)

        if DEBUG_PHASE == 5:
            with tc.tile_pool(name="dbg", bufs=2) as dbg:
                t = dbg.tile([P, D], DT, tag="dbg")
                nc.vector.tensor_copy(t, gw_sb[:, 0, :].bitcast(DT))
                nc.sync.dma_start(out=out_d.ap()[0:P, :], in_=t)

        # ---------------- Phase D: S1, K1, fused projection ----------------
        with tc.tile_pool(name="pd", bufs=3) as pd, \
             tc.tile_pool(name="pd_k1", bufs=2) as pdk1, \
             tc.tile_pool(name="pdp_s1", bufs=2, space="PSUM") as pdps1, \
             tc.tile_pool(name="pdp_t", bufs=2, space="PSUM") as pdpt, \
             tc.tile_pool(name="pdp_o", bufs=2, space="PSUM") as pdpo:
            ncount = NCH if DEBUG_PHASE != 8 else 1
            for ncx in range(ncount if RUN(6) else 0):
                nsl = slice(ncx * P, (ncx + 1) * P)
                s1 = pdps1.tile([P, HG * M], DT, tag="s1")
                for pr in range(PAIRS):
                    nc.tensor.matmul(
                        s1[:, pr * P:(pr + 1) * P],
                        QT[:, pr, nsl], klt_blk[:, pr, :],
                        start=(pr == 0), stop=(pr == PAIRS - 1),
                        skip_group_check=True)
                e1 = pd.tile([P, HG * M], DT, tag="e1")
                nc.scalar.activation(out=e1, in_=s1, func=EXP, scale=SCALE)
                r1 = pd.tile([P, HG], DT, tag="r1")
                nc.vector.reduce_sum(
                    r1, e1.rearrange("p (h m) -> p h m", h=HG), axis=AX)
                rc = pd.tile([P, HG], DT, tag="rc")
                nc.vector.reciprocal(rc, r1)
                e1n = pd.tile([P, HG * M], F32R, tag="e1n")
                for h6 in range(HG):
                    nc.vector.tensor_scalar_mul(
                        e1n[:, h6 * M:(h6 + 1) * M],
                        e1[:, h6 * M:(h6 + 1) * M], rc[:, h6:h6 + 1])
                k1ts = []
                for kc in range(PAIRS):
                    tp = pdpt.tile([P, P], F32R, tag="k1tp")
                    nc.tensor.transpose(
                        tp, e1n[:, kc * P:(kc + 1) * P], identr)
                    k1t = pdk1.tile([P, P], F32R, tag=f"k1t{kc}")
                    nc.any.tensor_copy(out=k1t, in_=tp.bitcast(DT))
                    k1ts.append(k1t)
                po1 = pdpo.tile([P, 512], DT, tag="po1")
                po2 = pdpo.tile([P, 256], DT, tag="po2")
                for kc in range(PAIRS):
                    nc.tensor.matmul(po1, k1ts[kc], gw_sb[:, kc, 0:512],
                                     start=(kc == 0), stop=(kc == PAIRS - 1))
                    nc.tensor.matmul(po2, k1ts[kc], gw_sb[:, kc, 512:768],
                                     start=(kc == 0), stop=(kc == PAIRS - 1))
                ob = pd.tile([P, D], DT, tag="ob")
                nc.vector.tensor_add(ob[:, 0:512], po1, bp_bc[:, 0:512])
                nc.vector.tensor_add(ob[:, 512:768], po2, bp_bc[:, 512:768])
                nc.sync.dma_start(out=out_d.ap()[nsl, :], in_=ob)

    nc.compile()
    return nc


def get_program():
    global _PROGRAM
    if _PROGRAM is None:
        _PROGRAM = _build_program()
    return _PROGRAM


def shard_inputs(x, w_qkv, b_qkv, w_proj, b_proj):
    """Returns one in_map per core (core = 2*b + head_group)."""
    x = np.ascontiguousarray(x, dtype=np.float32)
    w_qkv = np.asarray(w_qkv, dtype=np.float32)
    b_qkv = np.asarray(b_qkv, dtype=np.float32)
    w_proj = np.asarray(w_proj, dtype=np.float32)
    b_proj = np.asarray(b_proj, dtype=np.float32)
    wq, wk, wv = w_qkv[:, 0:D], w_qkv[:, D:2 * D], w_qkv[:, 2 * D:3 * D]
    bq, bk, bv = b_qkv[0:D], b_qkv[D:2 * D], b_qkv[2 * D:3 * D]
    in_maps = []
    for core in range(8):
        b, hg = core // 2, core % 2
        cs = slice(hg * HG * HD, (hg + 1) * HG * HD)
        wqk_c = np.ascontiguousarray(
            np.concatenate([wq[:, cs], wk[:, cs]], axis=1))
        in_maps.append({
            "x": np.ascontiguousarray(x[b]),
            "wqk": wqk_c,
            "xl": np.ascontiguousarray(x[b][0:4096:65, :]),
            "wqk32": wqk_c.copy(),
            "bqk": np.ascontiguousarray(
                np.concatenate([bq[cs], bk[cs]])),
            "wv": np.ascontiguousarray(wv[:, cs]),
            "bv": np.ascontiguousarray(bv[cs]),
            "wp": np.ascontiguousarray(w_proj[cs.start:cs.stop, :]),
            "bph": np.ascontiguousarray(0.5 * b_proj),
        })
    return in_maps


def run_cores(in_maps, trace=False, **kw):
    from concourse import bass_utils
    nc = get_program()
    return bass_utils.run_bass_kernel_spmd(
        nc, in_maps, core_ids=list(range(8)), trace=trace, **kw)


def unshard_output(results):
    out = np.empty((B, N, D), dtype=np.float32)
    for b in range(B):
        out[b] = results[2 * b]["out"] + results[2 * b + 1]["out"]
    return out


def kernel(x, w_qkv, b_qkv, w_proj, b_proj):
    in_maps = shard_inputs(x, w_qkv, b_qkv, w_proj, b_proj)
    res = run_cores(in_maps)
    return unshard_output(res.results)


# revision 24
# speedup vs baseline: 1.3009x; 1.0191x over previous
# Nystrom attention TRN2 kernel (B=4, N=4096, D=768, H=12, m=64 landmarks).
#
# Sharding: 8 cores; core c handles batch b = c//2 and a 6-head group
# hg = c%2 (heads 6*hg .. 6*hg+5, organized as 3 adjacent pairs).
# Each core computes its heads' full contribution through w_proj plus half
# the proj bias; the host unshards by summing the two partials per batch.
#
# Per-core pipeline (all matmuls on PE, fp32 data, fp32r perf mode where the
# moving free dim is large):
#   A: stream x in 512-row blocks, PE-transpose to get D-major X^T, then
#      QT/KT head-major (pair-stacked on partitions) resident in SBUF and
#      V row-major streamed to a DRAM scratch buffer.
#   B: landmark gathers (stride-65 column slices), S2 -> K2 -> A = K2+eps*I,
#      Newton-Schulz inverse (34 iters, first 20 in bf16, dual X/Z iterate,
#      2-head block-diagonal batching).
#   C: S3 m-major (block-diag landmark lhsT), exp, PE-transpose chunks,
#      E3V accumulation, normalize by r3 -> K3V.
#   GT/GW: G^T = K3V^T Z, GW_h = G_h @ Wp_h (m x 768, pair-stacked).
#   D: per 128-row chunk: S1 row-major, exp, row-normalize, PE-transpose,
#      fused (K1 @ GW) projection, +bias/2, DMA out.

import numpy as np

B, N, D = 4, 4096, 768
H, M, HD = 12, 64, 64
HG = 6          # heads per core
PAIRS = 3       # head pairs per core
P = 128
NBLK = 256      # phase A/C n-block
NB = N // NBLK  # 8
NCH = N // P    # 32
SCALE = 0.125   # hd^-0.5
EPS = 1e-6
NS_ITERS = 32
NS_BF16 = 20
LSTRIDE = 65    # landmark stride: linspace(0,4095,64) == 65*arange(64)

_PROGRAM = None
DEBUG_PHASE = 0  # 0=full, 1=A only, 2=A+B, 3=A+B+NS, 4=+C, 5=+GT/GW


def _build_program():
    import concourse.bass as bass
    import concourse.mybir as mybir
    import concourse.tile as tile
    from concourse import bacc
    from concourse.masks import make_identity
    from contextlib import ExitStack

    DT = mybir.dt.float32
    BF = mybir.dt.bfloat16
    F32R = mybir.dt.float32r
    EXP = mybir.ActivationFunctionType.Exp
    IDENT = mybir.ActivationFunctionType.Identity
    AX = mybir.AxisListType.X

    def r32(ap):
        return ap.bitcast(F32R)

    nc = bacc.Bacc(trn_type="TRN2", target_bir_lowering=False, debug=False)

    x_d = nc.dram_tensor("x", [N, D], F32R, kind="ExternalInput")
    wqk_d = nc.dram_tensor("wqk", [D, 2 * HG * HD], F32R, kind="ExternalInput")
    bqk_d = nc.dram_tensor("bqk", [2 * HG * HD], DT, kind="ExternalInput")
    wv_d = nc.dram_tensor("wv", [D, HG * HD], F32R, kind="ExternalInput")
    bv_d = nc.dram_tensor("bv", [HG * HD], DT, kind="ExternalInput")
    wp_d = nc.dram_tensor("wp", [HG * HD, D], F32R, kind="ExternalInput")
    bph_d = nc.dram_tensor("bph", [D], DT, kind="ExternalInput")
    xl_d = nc.dram_tensor("xl", [M, D], DT, kind="ExternalInput")
    wqk32_d = nc.dram_tensor("wqk32", [D, 2 * HG * HD], DT, kind="ExternalInput")
    out_d = nc.dram_tensor("out", [N, D], DT, kind="ExternalOutput")

    with tile.TileContext(nc) as tc, ExitStack() as ctx:
        singles = ctx.enter_context(tc.tile_pool(name="singles", bufs=1))
        res = ctx.enter_context(tc.tile_pool(name="res", bufs=1))
        dram = ctx.enter_context(tc.tile_pool(name="dram", bufs=1, space="DRAM"))

        ident = singles.tile([P, P], DT)
        make_identity(nc, ident)
        identr = singles.tile([P, P], F32R)
        nc.vector.tensor_copy(identr, ident)
        twoI = singles.tile([P, P], DT)
        nc.vector.tensor_scalar_mul(twoI, ident, 2.0)
        epsI = singles.tile([P, M], DT)  # [eps*I64 ; eps*I64] stacked
        nc.vector.tensor_scalar_mul(epsI[0:M, :], ident[0:M, 0:M], EPS)
        nc.vector.tensor_scalar_mul(epsI[M:P, :], ident[M:P, M:P], EPS)

        biasqk = singles.tile([P, 2 * PAIRS], DT)
        nc.sync.dma_start(out=biasqk, in_=bqk_d.ap().rearrange("(c p) -> p c", p=P))
        bv_bc = singles.tile([P, HG * HD], DT)
        bv_ap = bv_d.ap()
        nc.sync.dma_start(
            out=bv_bc,
            in_=bass.AP(tensor=bv_ap.tensor, offset=bv_ap.offset,
                        ap=[[0, P], [1, HG * HD]]),
        )
        bp_bc = singles.tile([P, D], DT)
        bp_ap = bph_d.ap()
        nc.sync.dma_start(
            out=bp_bc,
            in_=bass.AP(tensor=bp_ap.tensor, offset=bp_ap.offset,
                        ap=[[0, P], [1, D]]),
        )
        wqk_sb = singles.tile([P, 6, 2 * HG * HD], F32R)
        nc.sync.dma_start(out=wqk_sb, in_=wqk_d.ap().rearrange("(c p) f -> p c f", p=P))
        wv_sb = singles.tile([P, 6, HG * HD], F32R)
        nc.sync.dma_start(out=wv_sb, in_=wv_d.ap().rearrange("(c p) f -> p c f", p=P))
        wp_sb = singles.tile([M, HG, D], F32R)
        nc.sync.dma_start(out=wp_sb, in_=wp_d.ap().rearrange("(h p) f -> p h f", p=M))

        QT = res.tile([P, PAIRS, N], F32R)   # partitions = pair-stacked head dims
        KT = res.tile([P, PAIRS, N], F32R)
        gw_sb = res.tile([P, PAIRS, D], F32R)
        v_dram = dram.tile([N, HG * HD], F32R)

        # ---------------- Phase A: qkv projection ----------------
        with tc.tile_pool(name="pa", bufs=3) as pa, \
             tc.tile_pool(name="pa_vt", bufs=3) as pavt, \
             tc.tile_pool(name="pap_t", bufs=3, space="PSUM") as papt, \
             tc.tile_pool(name="pap_qk", bufs=3, space="PSUM") as papqk, \
             tc.tile_pool(name="pap_v", bufs=2, space="PSUM") as papv:
            for nb in range(NB):
                nsl = slice(nb * NBLK, (nb + 1) * NBLK)
                xin = pa.tile([P, NBLK // P, D], F32R, tag="xin")
                nc.sync.dma_start(
                    out=xin, in_=x_d.ap()[nsl, :].rearrange("(c p) d -> p c d", p=P))
                xt = pa.tile([P, 6, NBLK], F32R, tag="xt")
                for c4 in range(NBLK // P):
                    for dc in range(6):
                        tp = papt.tile([P, P], F32R, tag="tp")
                        nc.tensor.transpose(
                            tp, xin[:, c4, dc * P:(dc + 1) * P], identr)
                        nc.any.tensor_copy(
                            out=xt[:, dc, c4 * P:(c4 + 1) * P],
                            in_=tp.bitcast(DT))
                if DEBUG_PHASE == 7 and nb == 0:
                    with tc.tile_pool(name="dbg7", bufs=1) as dbg7:
                        t7 = dbg7.tile([P, D], DT)
                        for dc in range(3):
                            nc.vector.tensor_copy(
                                t7[:, dc * NBLK:(dc + 1) * NBLK],
                                xt[:, dc, :].bitcast(DT))
                        nc.sync.dma_start(out=out_d.ap()[0:P, :], in_=t7)
                        t8 = dbg7.tile([P, D], DT)
                        nc.vector.tensor_copy(t8[:, 0:D], xin[:, 0, :].bitcast(DT))
                        nc.sync.dma_start(out=out_d.ap()[P:2 * P, :], in_=t8)
                for oc in range(6):
                    ps = papqk.tile([P, NBLK], DT, tag="qk")
                    for dc in range(6):
                        nc.tensor.matmul(
                            ps, wqk_sb[:, dc, oc * P:(oc + 1) * P],
                            xt[:, dc, :],
                            start=(dc == 0), stop=(dc == 5))
                    dest = QT if oc < 3 else KT
                    col = oc if oc < 3 else oc - 3
                    nc.scalar.activation(
                        out=dest[:, col, nsl], in_=ps, func=IDENT,
                        bias=biasqk[:, oc:oc + 1], scale=1.0)
                for c4 in range(NBLK // P):
                    psv = papv.tile([P, HG * HD], DT, tag="v")
                    for dc in range(6):
                        nc.tensor.matmul(
                            psv, xt[:, dc, c4 * P:(c4 + 1) * P],
                            wv_sb[:, dc, :],
                            start=(dc == 0), stop=(dc == 5))
                    vt = pavt.tile([P, HG * HD], F32R, tag="vt")
                    nc.vector.tensor_add(vt, psv, bv_bc)
                    nc.sync.dma_start(
                        out=v_dram[(nb * (NBLK // P) + c4) * P:(nb * (NBLK // P) + c4 + 1) * P, :],
                        in_=vt)

        if DEBUG_PHASE == 1:
            with tc.tile_pool(name="dbg", bufs=2) as dbg:
                for ncx in range(NCH):
                    t = dbg.tile([P, D], DT, tag="dbg")
                    nc.vector.memset(t, 0.0)
                    nc.vector.tensor_copy(t[:, 0:128], QT[:, 0, ncx * P:(ncx + 1) * P].bitcast(DT))
                    nc.vector.tensor_copy(t[:, 128:256], KT[:, 0, ncx * P:(ncx + 1) * P].bitcast(DT))
                    nc.sync.dma_start(out=out_d.ap()[ncx * P:(ncx + 1) * P, :], in_=t)

        # ---------------- Phase B: landmarks, A matrices ----------------
        RUN = lambda k: DEBUG_PHASE == 0 or DEBUG_PHASE >= k
        pb = ctx.enter_context(tc.tile_pool(name="pb", bufs=1))
        with tc.tile_pool(name="pbp", bufs=2, space="PSUM") as pbp, \
             tc.tile_pool(name="pbt", bufs=1) as pbt:
            # fp32 landmark path: A = K2 + eps*I must be fp32-exact because
            # cond(A) ~ 2e4 amplifies f32r rounding in the inverse.
            qlt_blk = pb.tile([P, PAIRS, P], F32R)  # block-diag Q_l^T per pair
            klt_blk = pb.tile([P, PAIRS, P], F32R)  # block-diag K_l^T per pair
            nc.vector.memset(qlt_blk.bitcast(DT), 0.0)
            nc.vector.memset(klt_blk.bitcast(DT), 0.0)
            xl = pbt.tile([M, D], DT)
            nc.sync.dma_start(out=xl, in_=xl_d.ap())
            wqk32 = pbt.tile([P, 6, 2 * HG * HD], DT)
            nc.sync.dma_start(
                out=wqk32,
                in_=wqk32_d.ap().rearrange("(c p) f -> p c f", p=P))
            xlt = pbt.tile([P, 6, M], DT)
            for dc in range(6 if RUN(2) else 0):
                tx = pbp.tile([P, M], DT, tag="xlt")
                nc.tensor.transpose(tx, xl[:, dc * P:(dc + 1) * P],
                                    ident[0:M, 0:M])
                nc.any.tensor_copy(out=xlt[:, dc, :], in_=tx)
            qkl32 = pbt.tile([P, 6, M], DT)  # oc 0-2: Q_l^T pairs, 3-5: K_l^T
            for oc in range(6 if RUN(2) else 0):
                pql = pbp.tile([P, M], DT, tag="pql")
                for dc in range(6):
                    nc.tensor.matmul(pql, wqk32[:, dc, oc * P:(oc + 1) * P],
                                     xlt[:, dc, :],
                                     start=(dc == 0), stop=(dc == 5))
                nc.scalar.activation(out=qkl32[:, oc, :], in_=pql, func=IDENT,
                                     bias=biasqk[:, oc:oc + 1], scale=1.0)
            qlt_blk32 = pbt.tile([P, PAIRS, P], DT)
            nc.vector.memset(qlt_blk32, 0.0)
            for pr in range(PAIRS if RUN(2) else 0):
                nc.any.tensor_copy(out=klt_blk[0:M, pr, 0:M],
                                   in_=qkl32[0:M, 3 + pr, :])
                nc.any.tensor_copy(out=klt_blk[M:P, pr, M:P],
                                   in_=qkl32[M:P, 3 + pr, :])
                nc.any.tensor_copy(out=qlt_blk[0:M, pr, 0:M],
                                   in_=qkl32[0:M, pr, :])
                nc.any.tensor_copy(out=qlt_blk[M:P, pr, M:P],
                                   in_=qkl32[M:P, pr, :])
                nc.any.tensor_copy(out=qlt_blk32[0:M, pr, 0:M],
                                   in_=qkl32[0:M, pr, :])
                nc.any.tensor_copy(out=qlt_blk32[M:P, pr, M:P],
                                   in_=qkl32[M:P, pr, :])
            A_st = pb.tile([P, PAIRS, M], DT)     # pair-stacked A = K2 + eps*I
            r2 = pb.tile([P, PAIRS], DT)
            for pr in range(PAIRS if RUN(2) else 0):
                ps2 = pbp.tile([P, M], DT, tag="s2")
                nc.tensor.matmul(ps2, qlt_blk32[:, pr, :], qkl32[:, 3 + pr, :],
                                 start=True, stop=True)
                e2 = pb.tile([P, M], DT, tag=f"e2_{pr}")
                nc.scalar.activation(out=e2, in_=ps2, func=EXP, scale=SCALE,
                                     accum_out=r2[:, pr:pr + 1])
                r2c = pb.tile([P, 1], DT, tag=f"r2c_{pr}")
                nc.vector.reciprocal(r2c, r2[:, pr:pr + 1])
                nc.vector.tensor_scalar_mul(A_st[:, pr, :], e2, r2c)
                nc.vector.tensor_add(A_st[:, pr, :], A_st[:, pr, :], epsI)

        if DEBUG_PHASE == 2:
            with tc.tile_pool(name="dbg", bufs=2) as dbg:
                t = dbg.tile([P, D], DT, tag="dbg")
                nc.vector.memset(t, 0.0)
                nc.vector.tensor_copy(t[:, 0:3 * M], A_st.bitcast(DT) if A_st.dtype != DT else A_st)
                nc.sync.dma_start(out=out_d.ap()[0:P, :], in_=t)

        # ---------------- Phase C: S3, E3V, K3V ----------------
        k3v_st = pb.tile([P, PAIRS, M], DT)
        with tc.tile_pool(name="pc", bufs=3) as pc, \
             tc.tile_pool(name="pcp_s3", bufs=2, space="PSUM") as pcps3, \
             tc.tile_pool(name="pcp_t", bufs=2, space="PSUM") as pcpt, \
             tc.tile_pool(name="pcp_acc", bufs=1, space="PSUM") as pcpacc:
            r3acc = pb.tile([P, PAIRS, NB], DT)
            k3vps = []
            for pr in range(PAIRS):
                acc = pcpacc.tile([P, HG * HD], DT, tag=f"acc{pr}")
                k3vps.append(acc)  # one bank per pair, 384 wide
            for nb in range(NB if RUN(4) else 0):
                nsl = slice(nb * NBLK, (nb + 1) * NBLK)
                vblk = pc.tile([P, NBLK // P, HG * HD], F32R, tag="vblk")
                nc.sync.dma_start(
                    out=vblk,
                    in_=v_dram[nsl, :].rearrange("(c p) f -> p c f", p=P))
                for pr in range(PAIRS):
                    s3 = pcps3.tile([P, NBLK], DT, tag="s3")
                    nc.tensor.matmul(s3, qlt_blk[:, pr, :],
                                     KT[:, pr, nsl], start=True, stop=True)
                    e3 = pc.tile([P, NBLK], F32R, tag="e3")
                    nc.scalar.activation(out=e3, in_=s3, func=EXP, scale=SCALE,
                                         accum_out=r3acc[:, pr, nb:nb + 1])
                    for c4 in range(NBLK // P):
                        tp = pcpt.tile([P, P], F32R, tag="e3t")
                        nc.tensor.transpose(
                            tp, e3[:, c4 * P:(c4 + 1) * P], identr)
                        e3t = pc.tile([P, P], F32R, tag="e3ts")
                        nc.any.tensor_copy(out=e3t, in_=tp.bitcast(DT))
                        first = (nb == 0 and c4 == 0)
                        last = (nb == NB - 1 and c4 == NBLK // P - 1)
                        nc.tensor.matmul(
                            k3vps[pr], e3t, vblk[:, c4, :],
                            start=first, stop=last, skip_group_check=True)
            for pr in range(PAIRS if RUN(4) else 0):
                r3 = pb.tile([P, 1], DT, tag=f"r3_{pr}")
                nc.vector.reduce_sum(r3, r3acc[:, pr, :], axis=AX)
                r3c = pb.tile([P, 1], DT, tag=f"r3c_{pr}")
                nc.vector.reciprocal(r3c, r3)
                for h2 in range(2):
                    sl = slice(h2 * M, (h2 + 1) * M)
                    c0 = pr * P + h2 * M
                    nc.vector.tensor_scalar_mul(
                        k3v_st[sl, pr, :], k3vps[pr][sl, c0:c0 + M],
                        r3c[sl])

        # ---------------- Newton-Schulz inverse (dual iterate) ----------------
        ns_x = []
        ns_z = []
        with tc.tile_pool(name="nsp", bufs=2, space="PSUM") as nsp:
            for pr in range(PAIRS if RUN(3) else 0):
                Ablk = pb.tile([P, P], DT, tag=f"ablk{pr}")
                nc.vector.memset(Ablk, 0.0)
                nc.any.tensor_copy(out=Ablk[0:M, 0:M], in_=A_st[0:M, pr, :])
                nc.any.tensor_copy(out=Ablk[M:P, M:P], in_=A_st[M:P, pr, :])
                tb = nsp.tile([P, P], DT, tag=f"ns{pr}")
                nc.tensor.transpose(tb, Ablk, ident)  # A^T blockdiag
                b_f = pb.tile([P, P], DT, tag=f"bf{pr}")
                nc.any.tensor_copy(out=b_f, in_=tb)
                b_bf = pb.tile([P, P], BF, tag=f"bbf{pr}")
                nc.any.tensor_copy(out=b_bf, in_=tb)
                x_bf = pb.tile([P, P], BF, tag=f"xbf{pr}")
                nc.any.tensor_copy(out=x_bf, in_=tb)        # X0 = A^T (a0=1)
                z_bf = pb.tile([P, P], BF, tag=f"zbf{pr}")
                nc.any.tensor_copy(out=z_bf, in_=Ablk)      # Z0 = X0^T = A
                x_f = pb.tile([P, P], DT, tag=f"xf{pr}")
                z_f = pb.tile([P, P], DT, tag=f"zf{pr}")
                for it in range(NS_ITERS):
                    bf_now = it < NS_BF16
                    bf_next = (it + 1) < NS_BF16
                    cx = x_bf if bf_now else x_f
                    cz = z_bf if bf_now else z_f
                    cb = b_bf if bf_now else b_f
                    t1 = nsp.tile([P, P], DT, tag=f"ns{pr}")
                    nc.tensor.matmul(t1, cb, cx, start=True, stop=True)  # A@X
                    u = pb.tile([P, P], BF if bf_now else DT,
                                tag=f"u{pr}_{it % 2}_{int(bf_now)}")
                    nc.vector.tensor_sub(u, twoI, t1)        # U = 2I - A X
                    xn = nsp.tile([P, P], DT, tag=f"ns{pr}")
                    nc.tensor.matmul(xn, cz, u, start=True, stop=True)   # X@U
                    zn = nsp.tile([P, P], DT, tag=f"ns{pr}")
                    nc.tensor.matmul(zn, u, cz, start=True, stop=True)   # U^T@Z
                    if bf_next:
                        nc.any.tensor_copy(out=x_bf, in_=xn)
                        nc.any.tensor_copy(out=z_bf, in_=zn)
                    else:
                        nc.any.tensor_copy(out=x_f, in_=xn)
                        nc.any.tensor_copy(out=z_f, in_=zn)
                ns_x.append(x_f)
                ns_z.append(z_f)

        if DEBUG_PHASE == 3:
            with tc.tile_pool(name="dbg", bufs=2) as dbg:
                t = dbg.tile([P, D], DT, tag="dbg")
                nc.vector.memset(t, 0.0)
                for pr in range(PAIRS):
                    nc.vector.tensor_copy(t[:, pr * P:(pr + 1) * P], ns_z[pr])
                nc.sync.dma_start(out=out_d.ap()[0:P, :], in_=t)

        if DEBUG_PHASE == 4:
            with tc.tile_pool(name="dbg", bufs=2) as dbg:
                t = dbg.tile([P, D], DT, tag="dbg")
                nc.vector.memset(t, 0.0)
                nc.vector.tensor_copy(t[:, 0:3 * M], k3v_st)
                nc.sync.dma_start(out=out_d.ap()[0:P, :], in_=t)

        # ---------------- GT / GW ----------------
        with tc.tile_pool(name="pg", bufs=1) as pg, \
             tc.tile_pool(name="pgp", bufs=2, space="PSUM") as pgp:
            for pr in range(PAIRS if RUN(5) else 0):
                gtp = pgp.tile([M, P], DT, tag="gt")
                nc.tensor.matmul(gtp, k3v_st[:, pr, :], ns_z[pr],
                                 start=True, stop=True)  # [GT_h1 | GT_h2]
                gt = pg.tile([M, P], F32R, tag=f"gt{pr}")
                nc.any.tensor_copy(out=gt, in_=gtp)
                for h2 in range(2):
                    lt = gt[:, h2 * M:(h2 + 1) * M]
                    rh = wp_sb[:, 2 * pr + h2, :]
                    g1 = pgp.tile([M, 512], DT, tag="gw1")
                    nc.tensor.matmul(g1, lt, rh[:, 0:512],
                                     start=True, stop=True)
                    g2 = pgp.tile([M, 256], DT, tag="gw2")
                    nc.tensor.matmul(g2, lt, rh[:, 512:768],
                                     start=True, stop=True)
                    nc.any.tensor_copy(
                        out=gw_sb[h2 * M:(h2 + 1) * M, pr, 0:512], in_=g1)
                    nc.any.tensor_copy(
                        out=gw_sb[h2 * M:(h2 + 1) * M, pr, 512:768], in_=g2)

        if DEBUG_PHASE == 5:
            with tc.tile_pool(name="dbg", bufs=2) as dbg:
                t = dbg.tile([P, D], DT, tag="dbg")
                nc.vector.tensor_copy(t, gw_sb[:, 0, :].bitcast(DT))
                nc.sync.dma_start(out=out_d.ap()[0:P, :], in_=t)

        # ---------------- Phase D: S1, K1, fused projection ----------------
        with tc.tile_pool(name="pd", bufs=3) as pd, \
             tc.tile_pool(name="pd_k1", bufs=2) as pdk1, \
             tc.tile_pool(name="pdp_s1", bufs=2, space="PSUM") as pdps1, \
             tc.tile_pool(name="pdp_t", bufs=2, space="PSUM") as pdpt, \
             tc.tile_pool(name="pdp_o", bufs=2, space="PSUM") as pdpo:
            ncount = NCH if DEBUG_PHASE != 8 else 1
            for ncx in range(ncount if RUN(6) else 0):
                nsl = slice(ncx * P, (ncx + 1) * P)
                s1 = pdps1.tile([P, HG * M], DT, tag="s1")
                for pr in range(PAIRS):
                    nc.tensor.matmul(
                        s1[:, pr * P:(pr + 1) * P],
                        QT[:, pr, nsl], klt_blk[:, pr, :],
                        start=(pr == 0), stop=(pr == PAIRS - 1),
                        skip_group_check=True)
                e1 = pd.tile([P, HG * M], DT, tag="e1")
                nc.scalar.activation(out=e1, in_=s1, func=EXP, scale=SCALE)
                r1 = pd.tile([P, HG], DT, tag="r1")
                nc.vector.reduce_sum(
                    r1, e1.rearrange("p (h m) -> p h m", h=HG), axis=AX)
                rc = pd.tile([P, HG], DT, tag="rc")
                nc.vector.reciprocal(rc, r1)
                e1n = pd.tile([P, HG * M], F32R, tag="e1n")
                for h6 in range(HG):
                    nc.vector.tensor_scalar_mul(
                        e1n[:, h6 * M:(h6 + 1) * M],
                        e1[:, h6 * M:(h6 + 1) * M], rc[:, h6:h6 + 1])
                k1ts = []
                for kc in range(PAIRS):
                    tp = pdpt.tile([P, P], F32R, tag="k1tp")
                    nc.tensor.transpose(
                        tp, e1n[:, kc * P:(kc + 1) * P], identr)
                    k1t = pdk1.tile([P, P], F32R, tag=f"k1t{kc}")
                    nc.any.tensor_copy(out=k1t, in_=tp.bitcast(DT))
                    k1ts.append(k1t)
                po1 = pdpo.tile([P, 512], DT, tag="po1")
                po2 = pdpo.tile([P, 256], DT, tag="po2")
                for kc in range(PAIRS):
                    nc.tensor.matmul(po1, k1ts[kc], gw_sb[:, kc, 0:512],
                                     start=(kc == 0), stop=(kc == PAIRS - 1))
                    nc.tensor.matmul(po2, k1ts[kc], gw_sb[:, kc, 512:768],
                                     start=(kc == 0), stop=(kc == PAIRS - 1))
                ob = pd.tile([P, D], DT, tag="ob")
                nc.vector.tensor_add(ob[:, 0:512], po1, bp_bc[:, 0:512])
                nc.vector.tensor_add(ob[:, 512:768], po2, bp_bc[:, 512:768])
                nc.sync.dma_start(out=out_d.ap()[nsl, :], in_=ob)

    nc.compile()
    return nc


def get_program():
    global _PROGRAM
    if _PROGRAM is None:
        _PROGRAM = _build_program()
    return _PROGRAM


def shard_inputs(x, w_qkv, b_qkv, w_proj, b_proj):
    """Returns one in_map per core (core = 2*b + head_group)."""
    x = np.ascontiguousarray(x, dtype=np.float32)
    w_qkv = np.asarray(w_qkv, dtype=np.float32)
    b_qkv = np.asarray(b_qkv, dtype=np.float32)
    w_proj = np.asarray(w_proj, dtype=np.float32)
    b_proj = np.asarray(b_proj, dtype=np.float32)
    wq, wk, wv = w_qkv[:, 0:D], w_qkv[:, D:2 * D], w_qkv[:, 2 * D:3 * D]
    bq, bk, bv = b_qkv[0:D], b_qkv[D:2 * D], b_qkv[2 * D:3 * D]
    in_maps = []
    for core in range(8):
        b, hg = core // 2, core % 2
        cs = slice(hg * HG * HD, (hg + 1) * HG * HD)
        wqk_c = np.ascontiguousarray(
            np.concatenate([wq[:, cs], wk[:, cs]], axis=1))
        in_maps.append({
            "x": np.ascontiguousarray(x[b]),
            "wqk": wqk_c,
            "xl": np.ascontiguousarray(x[b][0:4096:65, :]),
            "wqk32": wqk_c.copy(),
            "bqk": np.ascontiguousarray(
                np.concatenate([bq[cs], bk[cs]])),
            "wv": np.ascontiguousarray(wv[:, cs]),
            "bv": np.ascontiguousarray(bv[cs]),
            "wp": np.ascontiguousarray(w_proj[cs.start:cs.stop, :]),
            "bph": np.ascontiguousarray(0.5 * b_proj),
        })
    return in_maps


def run_cores(in_maps, trace=False, **kw):
    from concourse import bass_utils
    nc = get_program()
    return bass_utils.run_bass_kernel_spmd(
        nc, in_maps, core_ids=list(range(8)), trace=trace, **kw)


def unshard_output(results):
    out = np.empty((B, N, D), dtype=np.float32)
    for b in range(B):
        out[b] = results[2 * b]["out"] + results[2 * b + 1]["out"]
    return out


def kernel(x, w_qkv, b_qkv, w_proj, b_proj):
    in_maps = shard_inputs(x, w_qkv, b_qkv, w_proj, b_proj)
    res = run_cores(in_maps)
    return unshard_output(res.results)
